# revision 33
# baseline (speedup 1.0000x reference)
"""SAM-block (windowed attention + MLP) + FRM fusion on 8 TRN2 NeuronCores.

v2: bf16 datapath (fp32 PSUM accumulation), SBUF-resident intermediates,
batched rel-pos gather DMAs, per-window score matmuls, bf16 outputs.

Self-contained: shards the 100 attention windows over 8 cores (13/core),
runs one SPMD Bass program via run_bass_kernel_spmd, reassembles on host.
Device layout: [C(partitions), tokens(free)] for projections; attention
blocks operate on window pairs (392 tokens) with tokens on partitions.
"""
import os
import numpy as np
import ml_dtypes
import concourse.bass as bass
import concourse.bacc as bacc
import concourse.mybir as mybir
from concourse import tile
from concourse.masks import make_identity
from concourse.bass_utils import run_bass_kernel_spmd

F32 = mybir.dt.float32
BF = mybir.dt.bfloat16
F8 = mybir.dt.float8e4
AF = mybir.ActivationFunctionType
OP = mybir.AluOpType
NPBF = ml_dtypes.bfloat16

# problem constants
B, HH, WW, C = 4, 64, 64, 768
WIN, NH, HD = 14, 12, 64
S = WIN
N = S * S                  # 196 tokens / window
GRID = 5                   # 5x5 windows per image (64 -> 70 padded)
NWIN_TOT = B * GRID * GRID  # 100
NCORE = 8
NW = 13                    # windows per core (104 slots, 4 dummy)
NTOK = NW * N              # 2548
NT = 2 * N                 # 392 = pair block
NCH = C // 128             # 6
DFF = 4 * C
SCALE = HD ** -0.5
EPS = 1e-6
# 6 pair blocks + 1 single-window block (window 12)
BLOCKS = [(i * NT, 2) for i in range(6)] + [(6 * NT, 1)]
P2_TILES = [392] * 6 + [196]
P34_TILES = [392] * 6 + [196]

_CACHE = {}


def _build():
    nc = bacc.Bacc("TRN2", target_bir_lowering=False, debug=False)
    dt_in = {}

    def din(name, shape, dt=BF):
        dt_in[name] = nc.dram_tensor(name, shape, dt, kind="ExternalInput")
        return dt_in[name]

    xT_d = din("xT", [C, NTOK])
    rgbT_d = din("rgbT", [C, NTOK])
    mask_d = din("mask", [1, NTOK], BF)
    imgmask_d = din("imgmask", [4, NW], F32)
    imgneg_d = din("imgneg", [4, NW], F32)
    imgsel_d = din("imgsel", [4, NTOK], BF)
    qkwT_d = din("qkwT", [C, 2 * C])
    qkb_d = din("qkb", [2 * C], F32)
    vwT_d = din("vwT", [C, C])
    vb_d = din("vb", [C], F32)
    projwT_d = din("projwT", [C, C])
    projb_d = din("projb", [C], F32)
    n1w_d = din("n1w", [C], F32)
    n1b_d = din("n1b", [C], F32)
    n2w_d = din("n2w", [C], F32)
    n2b_d = din("n2b", [C], F32)
    tab_d = din("tab", [HD, 54])
    fc1w8_d = din("fc1w8", [128, 3, 2, DFF], F8)
    fc1b_d = din("fc1b", [DFF], F32)
    fc2wT_d = din("fc2wT", [DFF, C])
    fc2b_d = din("fc2b", [C], F32)
    sw1wT_d = din("sw1wT", [2 * C, C])
    sw1b_d = din("sw1b", [C], F32)
    sw2wT_d = din("sw2wT", [C, 2])
    sw2b_d = din("sw2b", [2], F32)
    cw1wTs_d = din("cw1wTs", [4 * C, 4 * C // NCORE])
    cw1bs_d = din("cw1bs", [4 * C // NCORE], F32)
    cw2wTs_d = din("cw2wTs", [4 * C, 2 * C // NCORE])
    cw2bs_d = din("cw2bs", [2 * C // NCORE], F32)
    out1_d = nc.dram_tensor("out1T", [C, NTOK], BF, kind="ExternalOutput")
    out2_d = nc.dram_tensor("out2T", [C, NTOK], BF, kind="ExternalOutput")

    core_ids = list(range(NCORE))
    r6 = lambda ap: ap.rearrange("(c p) n -> p c n", p=128)
    rcol = lambda ap: ap.rearrange("(c p) -> p c", p=128)

    with tile.TileContext(nc) as tc:
      with tc.tile_pool(name="dram", bufs=1, space="DRAM") as dramp, \
           tc.tile_pool(name="fbp", bufs=2, space="DRAM") as fbp, \
           tc.tile_pool(name="cst", bufs=1) as cp, \
           tc.tile_pool(name="res", bufs=1) as resp, \
           tc.tile_pool(name="stg", bufs=2) as stg:
        # ---------- DRAM scratch (collectives only) ----------
        csum_in = dramp.tile([128, 48], F32)
        csum_out = dramp.tile([128, 48], F32, addr_space="Shared")
        cmax_in = dramp.tile([128, 48], F32)
        cmax_out = dramp.tile([128, 48], F32, addr_space="Shared")
        z1_in = dramp.tile([4 * C // NCORE, 4], F32)
        z1_out = dramp.tile([4 * C, 4], F32, addr_space="Shared")
        z2_in = dramp.tile([2 * C // NCORE, 4], F32)
        z2_out = dramp.tile([2 * C, 4], F32, addr_space="Shared")

        # ---------- SBUF-resident activations ----------
        # x_all holds x -> (in-place) x2a = x+attn -> (in-place) x2 = x2a+mlp
        x_all = resp.tile([128, NCH, NTOK], BF)
        nc.sync.dma_start(x_all[:], r6(xT_d[:]))
        rgb_all = resp.tile([128, NCH, NTOK], BF)     # rgb input

        # ---------- persistent constants ----------
        identf = stg.tile([128, 128], F32, tag="st1")
        make_identity(nc, identf)
        ident98 = cp.tile([98, 98], BF)
        nc.vector.tensor_copy(ident98[:], identf[0:98, 0:98])

        def load_rows(src, n=C):
            t = cp.tile([128, n // 128], F32, name="rows_" + src.tensor.name)
            nc.sync.dma_start(t[:], rcol(src))
            return t

        qkb_t = load_rows(qkb_d[:], 2 * C)
        projb_t = load_rows(projb_d[:])
        n1w_t = load_rows(n1w_d[:])
        n1b_t = load_rows(n1b_d[:])
        n2w_t = load_rows(n2w_d[:])
        n2b_t = load_rows(n2b_d[:])
        fc1b_t = load_rows(fc1b_d[:], DFF)
        fc2b_t = load_rows(fc2b_d[:])
        sw1b_t = load_rows(sw1b_d[:])
        sw2b_t = cp.tile([2, 1], F32)
        nc.sync.dma_start(sw2b_t[:, 0], sw2b_d[:])
        cw1bs_t = load_rows(cw1bs_d[:], 4 * C // NCORE)
        cw2bs_t = cp.tile([128, 2], F32)
        nc.any.memset(cw2bs_t[:], 0.0)
        nc.sync.dma_start(cw2bs_t[0:128, 0], cw2bs_d[0:128])
        nc.sync.dma_start(cw2bs_t[0:64, 1], cw2bs_d[128:192])
        vb_t = load_rows(vb_d[:])
        tab2 = cp.tile([128, 2, 54], BF)
        nc.any.memset(tab2[:], 0.0)
        nc.sync.dma_start(tab2[0:64, 0, :], tab_d[:])
        nc.sync.dma_start(tab2[64:128, 1, :], tab_d[:])
        ones_f = stg.tile([128, 1], F32, tag="st1")
        nc.any.memset(ones_f[:], 1.0)
        ones_r = cp.tile([128, 1], BF)
        nc.vector.tensor_copy(ones_r[:], ones_f[:])
        ones_row = cp.tile([1, 128], BF)
        nc.any.memset(ones_row[:], 1.0)
        mask_bf = cp.tile([1, NTOK], BF)
        nc.sync.dma_start(mask_bf[:], mask_d[:])

        def load_w(pool_, shape3, src_ap, nm, dt=BF):
            # weights load directly (no staging/round)
            r = pool_.tile(shape3, dt, name="w_" + nm)
            nc.sync.dma_start(r[:], src_ap)
            return r

        # ==================== PHASE 1: attention ====================
        with tc.tile_pool(name="w1", bufs=1) as wp1, \
             tc.tile_pool(name="p1", bufs=1) as p1, \
             tc.tile_pool(name="p1b", bufs=1) as p1b, \
             tc.tile_pool(name="p1r", bufs=1) as p1r, \
             tc.tile_pool(name="p1s", bufs=2) as p1s, \
             tc.tile_pool(name="p1f", bufs=2) as p1f, \
             tc.tile_pool(name="ln", bufs=2, space="PSUM") as psln, \
             tc.tile_pool(name="gen", bufs=4, space="PSUM") as psg, \
             tc.tile_pool(name="pst", bufs=1, space="PSUM") as pst, \
             tc.tile_pool(name="psa", bufs=1, space="PSUM") as psa:
            qkwT = load_w(wp1, [128, NCH, 2 * C], r6(qkwT_d[:]), "qk")
            vwT = load_w(wp1, [128, NCH, C], r6(vwT_d[:]), "v")
            projwT = load_w(wp1, [128, NCH, C], r6(projwT_d[:]), "pj")

            kzAB = [p1.tile([128, NH, NT], BF, tag=f"kz{i}", name=f"kz{i}")
                    for i in range(2)]
            nc.any.memset(kzAB[0][:], 0.0)
            nc.any.memset(kzAB[1][:], 0.0)

            for blk, (c0, nw) in enumerate(BLOCKS):
                NTb = nw * N
                cols = slice(c0, c0 + NTb)
                kzt = kzAB[blk % 2]

                # ---- LN1 stats ----
                s1 = psln.tile([1, NT], F32, tag="lnp")
                s2 = psln.tile([1, NT], F32, tag="lnp")
                for c in range(NCH):
                    nc.tensor.matmul(s1[:, 0:NTb], ones_r[:], x_all[:, c, cols],
                                     start=(c == 0), stop=(c == NCH - 1))
                for c in range(NCH):
                    xsqc = p1f.tile([128, NT], BF, tag="xsqc", bufs=1)
                    nc.vector.tensor_mul(xsqc[:, 0:NTb], x_all[:, c, cols],
                                         x_all[:, c, cols])
                    nc.tensor.matmul(s2[:, 0:NTb], ones_r[:], xsqc[:, 0:NTb],
                                     start=(c == 0), stop=(c == NCH - 1))
                mrow_t = p1r.tile([1, NT], F32, tag="r1")
                nc.scalar.activation(mrow_t[:, 0:NTb], s1[:, 0:NTb], AF.Copy,
                                     scale=1.0 / C)
                var = p1r.tile([1, NT], F32, tag="r2")
                nc.vector.tensor_mul(var[:, 0:NTb], mrow_t[:, 0:NTb],
                                     mrow_t[:, 0:NTb])
                e2 = p1r.tile([1, NT], F32, tag="r3")
                nc.scalar.activation(e2[:, 0:NTb], s2[:, 0:NTb], AF.Copy,
                                     scale=1.0 / C)
                nc.vector.tensor_sub(var[:, 0:NTb], e2[:, 0:NTb], var[:, 0:NTb])
                nc.vector.tensor_scalar_add(var[:, 0:NTb], var[:, 0:NTb], EPS)
                sd = p1r.tile([1, NT], F32, tag="r4")
                nc.scalar.activation(sd[:, 0:NTb], var[:, 0:NTb], AF.Sqrt)
                inv = p1r.tile([1, NT], F32, tag="r5")
                nc.vector.reciprocal(inv[:, 0:NTb], sd[:, 0:NTb])
                mrow_b = p1r.tile([1, NT], BF, tag="r6")
                nc.vector.tensor_copy(mrow_b[:, 0:NTb], mrow_t[:, 0:NTb])
                # fold the window-validity mask into inv (norm1_b == 0)
                inv_bf = p1r.tile([1, NT], BF, tag="r7")
                nc.vector.tensor_mul(inv_bf[:, 0:NTb], inv[:, 0:NTb],
                                     mask_bf[:, cols])
                m_b = p1f.tile([128, NT], BF, tag="m_b", bufs=1)
                nc.gpsimd.partition_broadcast(m_b[:, 0:NTb], mrow_b[:, 0:NTb])
                inv_b = p1f.tile([128, NT], BF, tag="inv_b", bufs=1)
                nc.gpsimd.partition_broadcast(inv_b[:, 0:NTb], inv_bf[:, 0:NTb])
                lnx = p1b.tile([128, NCH, NT], BF, tag="lnx")
                for c in range(NCH):
                    u = p1s.tile([128, NT], BF, tag="u")
                    nc.vector.tensor_sub(u[:, 0:NTb], x_all[:, c, cols],
                                         m_b[:, 0:NTb])
                    nc.vector.tensor_mul(u[:, 0:NTb], u[:, 0:NTb],
                                         inv_b[:, 0:NTb])
                    nc.scalar.activation(lnx[:, c, 0:NTb], u[:, 0:NTb],
                                         AF.Identity, bias=n1b_t[:, c:c + 1],
                                         scale=n1w_t[:, c:c + 1])

                # ---- qk projection ----
                qT = p1b.tile([128, NCH, NT], BF, tag="qT")
                for mc in range(12):
                    ps = psg.tile([128, NT], F32, tag="gen")
                    for kc in range(NCH):
                        nc.tensor.matmul(ps[:, 0:NTb],
                                         qkwT[:, kc, mc * 128:(mc + 1) * 128],
                                         lnx[:, kc, 0:NTb], start=(kc == 0),
                                         stop=(kc == NCH - 1))
                    if mc < NCH:
                        nc.scalar.activation(qT[:, mc, 0:NTb], ps[:, 0:NTb],
                                             AF.Identity,
                                             bias=qkb_t[:, mc:mc + 1],
                                             scale=SCALE)
                    else:
                        j = mc - NCH
                        nc.scalar.activation(kzt[0:64, 2 * j, 0:NTb],
                                             ps[0:64, 0:NTb], AF.Identity,
                                             bias=qkb_t[0:64, mc:mc + 1],
                                             scale=1.0)
                        nc.scalar.activation(kzt[64:128, 2 * j + 1, 0:NTb],
                                             ps[64:128, 0:NTb], AF.Identity,
                                             bias=qkb_t[64:128, mc:mc + 1],
                                             scale=1.0)

                # ---- f features (rel-pos), batched ----
                # layout [p, qc, w, h, t] so (w,h) is one contiguous dim
                fall = p1b.tile([98, 2, 2, NH, 54], BF, tag="fall",
                                name=f"fall_{blk}")
                for qc in range(2):
                    for w in range(nw):
                        ps1 = psg.tile([98, 9 * 54], F32, tag="gen")
                        ps2 = psg.tile([98, 3 * 54], F32, tag="gen")
                        for h in range(NH):
                            dst = ps1[:, h * 54:h * 54 + 54] if h < 9 else \
                                ps2[:, (h - 9) * 54:(h - 9) * 54 + 54]
                            lhsT = qT[:, h // 2, w * N + qc * 98: w * N + qc * 98 + 98]
                            nc.tensor.matmul(dst, lhsT, tab2[:, h % 2, :],
                                             start=True, stop=True)
                        nc.scalar.copy(fall[:, qc, w, 0:9, :],
                                       ps1[:].rearrange("p (h t) -> p h t", t=54))
                        nc.scalar.copy(fall[:, qc, w, 9:12, :],
                                       ps2[:].rearrange("p (h t) -> p h t", t=54))
                fd = fbp.tile([98, 2, 2, NH, 54], BF, tag="fb", name=f"fd_{blk}")
                nc.sync.dma_start(fd[:], fall[:])
                gh = p1f.tile([98, 2, 2, NH, 14], BF, tag="gh", name=f"gh_{blk}")
                gw = p1f.tile([98, 2, 2, NH, 14], BF, tag="gw", name=f"gw_{blk}")
                for qc in range(2):
                    for g in range(7):
                        ih = qc * 7 + g
                        nc.sync.dma_start(
                            gh[g * 14:(g + 1) * 14, qc, :, :, :],
                            fall[g * 14:(g + 1) * 14, qc, :, :,
                                 13 - ih:27 - ih])
                        src_ap = bass.AP(fd[:].tensor,
                                         (g * 14) * 2592 + qc * 1296 + 40,
                                         [[2591, 14], [54, 2 * NH], [1, 14]])
                        nc.sync.dma_start(
                            gw[g * 14:(g + 1) * 14, qc, :, :, :].rearrange(
                                "p w h k -> p (w h) k"), src_ap)

                # ---- v (token-major pairs) ----
                v_pair = [p1b.tile([98, NH, 2, 64], BF, tag=f"vp{i}",
                                   name=f"vp{i}_{blk}") for i in range(2)]
                for c4 in range(2 * nw):
                    ps_a = psg.tile([98, 512], F32, tag="gen")
                    ps_b = psg.tile([98, 256], F32, tag="gen")
                    for kc in range(NCH):
                        nc.tensor.matmul(ps_a[:], lnx[:, kc, c4 * 98:(c4 + 1) * 98],
                                         vwT[:, kc, 0:512], start=(kc == 0),
                                         stop=(kc == NCH - 1))
                    for kc in range(NCH):
                        nc.tensor.matmul(ps_b[:], lnx[:, kc, c4 * 98:(c4 + 1) * 98],
                                         vwT[:, kc, 512:768], start=(kc == 0),
                                         stop=(kc == NCH - 1))
                    nc.scalar.copy(
                        v_pair[c4 % 2][:, 0:8, c4 // 2, :],
                        ps_a[:].rearrange("p (h d) -> p h d", d=64))
                    nc.scalar.copy(
                        v_pair[c4 % 2][:, 8:12, c4 // 2, :],
                        ps_b[:].rearrange("p (h d) -> p h d", d=64))

                # ---- scores + softmax + transpose + AV ----
                attn_outT = p1b.tile([128, NCH, NT], BF, tag="attn_outT")
                GH = 2
                for grp in range(NH // GH):
                    P_rg = {}
                    for w in range(nw):
                        for qc in range(2):
                            sco = p1f.tile([98, GH, S, S], BF, tag="sco", bufs=3)
                            ps = psg.tile([98, GH, S, S], F32, tag="gen")
                            lhsT = qT[:, grp,
                                      w * N + qc * 98: w * N + qc * 98 + 98]
                            for hh in range(GH):
                                h = grp * GH + hh
                                nc.tensor.matmul(ps[:, hh, :, :], lhsT,
                                                 kzt[:, h, w * N:(w + 1) * N],
                                                 start=True, stop=True)
                            nc.vector.tensor_add(
                                sco[:], ps[:],
                                gh[:, qc, w, grp * GH:grp * GH + GH, :, None]
                                .broadcast_to([98, GH, S, S]))
                            nc.vector.tensor_add(
                                sco[:], sco[:],
                                gw[:, qc, w, grp * GH:grp * GH + GH, None, :]
                                .broadcast_to([98, GH, S, S]))
                            nc.scalar.activation(sco[:], sco[:], AF.Exp)
                            z = p1s.tile([98, GH, 1, 1], F32, tag="z", bufs=3)
                            nc.vector.tensor_reduce(z[:], sco[:],
                                                    mybir.AxisListType.XY, OP.add)
                            nc.vector.reciprocal(z[:], z[:])
                            prg = p1f.tile([98, GH, S, S], BF, tag=f"P{w}{qc}",
                                           name=f"P{w}{qc}_{blk}_{grp}",
                                           bufs=3)
                            for hh in range(GH):
                                nc.vector.tensor_scalar_mul(
                                    prg[:, hh, :, :], sco[:, hh, :, :],
                                    z[:, hh, 0, :])
                            P_rg[(w, qc)] = prg
                    for hh in range(GH):
                        h = grp * GH + hh
                        PT_sb = []
                        for kc in range(2):
                            psT = pst.tile([98, 392], BF, tag="psT")
                            for j, (w, qc) in enumerate(
                                    [(w, qc) for w in range(nw)
                                     for qc in range(2)]):
                                nc.tensor.transpose(
                                    psT[:, j * 98:(j + 1) * 98],
                                    P_rg[(w, qc)][:, hh, 7 * kc:7 * kc + 7, :],
                                    ident98[:])
                            sb = p1s.tile([98, 392], BF, tag="PTsb")
                            nc.scalar.copy(sb[:, 0:nw * N], psT[:, 0:nw * N])
                            PT_sb.append(sb)
                        psA = psa.tile([128, 392], F32, tag="psAV")
                        for kc in range(2):
                            nc.tensor.matmul(psA[0:64 * nw, 0:NTb],
                                             v_pair[kc][:, h, 0:nw, :],
                                             PT_sb[kc][:, 0:NTb], start=(kc == 0),
                                             stop=(kc == 1))
                        nc.scalar.activation(
                            attn_outT[(h % 2) * 64:(h % 2) * 64 + 64, h // 2,
                                      0:196],
                            psA[0:64, 0:196], AF.Identity,
                            bias=vb_t[(h % 2) * 64:(h % 2) * 64 + 64,
                                      h // 2:h // 2 + 1], scale=1.0)
                        if nw == 2:
                            nc.vector.tensor_scalar_add(
                                attn_outT[(h % 2) * 64:(h % 2) * 64 + 64, h // 2,
                                          196:392],
                                psA[64:128, 196:392],
                                vb_t[(h % 2) * 64:(h % 2) * 64 + 64,
                                     h // 2:h // 2 + 1])

                # ---- proj + residual -> x2a (SBUF) ----
                for mc in range(NCH):
                    ps = psg.tile([128, NT], F32, tag="gen")
                    for kc in range(NCH):
                        nc.tensor.matmul(ps[:, 0:NTb],
                                         projwT[:, kc, mc * 128:(mc + 1) * 128],
                                         attn_outT[:, kc, 0:NTb], start=(kc == 0),
                                         stop=(kc == NCH - 1))
                    tmp = p1s.tile([128, NT], BF, tag="projtmp")
                    nc.scalar.activation(tmp[:, 0:NTb], ps[:, 0:NTb], AF.Identity,
                                         bias=projb_t[:, mc:mc + 1], scale=1.0)
                    nc.vector.tensor_add(x_all[:, mc, cols], tmp[:, 0:NTb],
                                         x_all[:, mc, cols])

        # ============ PHASE 2: LN2 stat pre-pass + fp8 MLP ============
        m2_row = resp.tile([1, NTOK], BF, name="m2_row")
        i2_row = resp.tile([1, NTOK], BF, name="i2_row")
        msk_all = resp.tile([128, NTOK], BF, name="msk_all")
        nc.gpsimd.partition_broadcast(msk_all[:], mask_bf[:])
        neg_all = resp.tile([128, NTOK], BF, name="neg_all")
        nc.vector.tensor_scalar_add(neg_all[:], msk_all[:], -1.0)
        nc.vector.tensor_scalar_mul(neg_all[:], neg_all[:], 1e30)
        W_s1 = resp.tile([128, NCH, 14, 1], F32, name="W_s1")
        W_s2 = resp.tile([128, NCH, 14, 1], F32, name="W_s2")
        W_m1 = resp.tile([128, NCH, 14, 1], F32, name="W_m1")
        W_m2 = resp.tile([128, NCH, 14, 1], F32, name="W_m2")
        with tc.tile_pool(name="pre2", bufs=2) as pre2, \
             tc.tile_pool(name="psp", bufs=2, space="PSUM") as psp:
            # prefetch rgb for stats fusion + phases 3/4
            nc.sync.dma_start(rgb_all[:], r6(rgbT_d[:]))
            col0 = 0
            for tl in P2_TILES:
                cs = slice(col0, col0 + tl)
                s1 = psp.tile([1, tl], F32, tag="s1", name=f"pps1_{col0}")
                s2 = psp.tile([1, tl], F32, tag="s2", name=f"pps2_{col0}")
                for c in range(NCH):
                    nc.tensor.matmul(s1[:], ones_r[:], x_all[:, c, cs],
                                     start=(c == 0), stop=(c == NCH - 1))
                for c in range(NCH):
                    xsqc = pre2.tile([128, tl], BF, tag="xsqc",
                                     name=f"xsq2_{col0}_{c}")
                    nc.vector.tensor_mul(xsqc[:], x_all[:, c, cs],
                                         x_all[:, c, cs])
                    nc.tensor.matmul(s2[:], ones_r[:], xsqc[:],
                                     start=(c == 0), stop=(c == NCH - 1))
                mrow = pre2.tile([1, tl], F32, tag="r1", name=f"m2m_{col0}")
                nc.scalar.activation(mrow[:], s1[:], AF.Copy, scale=1.0 / C)
                var = pre2.tile([1, tl], F32, tag="r2", name=f"m2v_{col0}")
                nc.vector.tensor_mul(var[:], mrow[:], mrow[:])
                e2 = pre2.tile([1, tl], F32, tag="r3", name=f"m2e_{col0}")
                nc.scalar.activation(e2[:], s2[:], AF.Copy, scale=1.0 / C)
                nc.vector.tensor_sub(var[:], e2[:], var[:])
                nc.vector.tensor_scalar_add(var[:], var[:], EPS)
                sd = pre2.tile([1, tl], F32, tag="r4", name=f"m2s_{col0}")
                nc.scalar.activation(sd[:], var[:], AF.Sqrt)
                inv = pre2.tile([1, tl], F32, tag="r5", name=f"m2i_{col0}")
                nc.vector.reciprocal(inv[:], sd[:])
                nc.vector.tensor_copy(m2_row[:, cs], mrow[:])
                nc.vector.tensor_copy(i2_row[:, cs], inv[:])
                col0 += tl
        with tc.tile_pool(name="w2", bufs=1) as wp2, \
             tc.tile_pool(name="p2", bufs=2) as p2, \
             tc.tile_pool(name="p2f", bufs=2) as p2f, \
             tc.tile_pool(name="ps1p", bufs=2, space="PSUM") as ps1p, \
             tc.tile_pool(name="ps2p", bufs=6, space="PSUM") as ps2p:
            fc1w8 = load_w(wp2, [128, 3, 2, DFF], fc1w8_d[:], "fc1", dt=F8)
            fc2wT = load_w(wp2, [128, DFF // 128, C], r6(fc2wT_d[:]), "fc2")
            DR = mybir.MatmulPerfMode.DoubleRow
            col0 = 0
            for ti, tl in enumerate(P2_TILES):
                nwt = tl // N
                cs = slice(col0, col0 + tl)
                m_b = p2f.tile([128, tl], BF, tag="m_b", name=f"mb_{col0}")
                nc.gpsimd.partition_broadcast(m_b[:], m2_row[:, cs])
                inv_b = p2f.tile([128, tl], BF, tag="inv_b", name=f"ib_{col0}")
                nc.gpsimd.partition_broadcast(inv_b[:], i2_row[:, cs])
                ln28 = p2.tile([128, 3, 2, tl], F8, tag="ln28", name=f"l2_{col0}")
                for c in range(NCH):
                    u = p2.tile([128, tl], BF, tag="u", name=f"u_{col0}_{c}")
                    nc.vector.tensor_sub(u[:], x_all[:, c, cs], m_b[:])
                    nc.vector.tensor_mul(u[:], u[:], inv_b[:])
                    nc.scalar.activation(ln28[:, c // 2, c % 2, :], u[:],
                                         AF.Identity, bias=n2b_t[:, c:c + 1],
                                         scale=n2w_t[:, c:c + 1])
                h_bf = p2.tile([128, 24, tl], BF, tag="h8", name=f"h8_{col0}")
                psum2 = [ps2p.tile([128, tl], F32, tag="ps2",
                                   name=f"ps2_{col0}_{m}") for m in range(NCH)]
                for kt2 in range(12):
                    for jj in range(2):
                        m24 = kt2 * 2 + jj
                        ps1 = ps1p.tile([128, tl], F32, tag="ps1",
                                        name=f"ps1_{col0}_{m24}")
                        for kt in range(3):
                            nc.tensor.matmul(
                                ps1[:], fc1w8[:, kt, :, m24 * 128:(m24 + 1) * 128],
                                ln28[:, kt, :, :], start=(kt == 0),
                                stop=(kt == 2), perf_mode=DR)
                        nc.scalar.activation(h_bf[:, m24, :], ps1[:], AF.Gelu,
                                             bias=fc1b_t[:, m24:m24 + 1],
                                             scale=1.0)
                        for mc in range(NCH):
                            nc.tensor.matmul(
                                psum2[mc][:],
                                fc2wT[:, m24, mc * 128:(mc + 1) * 128],
                                h_bf[:, m24, :], start=(m24 == 0),
                                stop=(m24 == 23))
                for mc in range(NCH):
                    tmp = p2.tile([128, tl], BF, tag="fct", name=f"fct_{col0}_{mc}")
                    nc.scalar.activation(tmp[:], psum2[mc][:], AF.Identity,
                                         bias=fc2b_t[:, mc:mc + 1], scale=1.0)
                    nc.vector.tensor_add(x_all[:, mc, cs], tmp[:],
                                         x_all[:, mc, cs])
                # FRM per-window stats fused here (x1 = rgb, x2 = x_all)
                for (xx, Ws, Wm) in ((rgb_all, W_s1, W_m1),
                                     (x_all, W_s2, W_m2)):
                    for c in range(NCH):
                        xm = p2f.tile([128, tl], BF, tag="xm",
                                      name=f"xm2_{col0}_{Ws.name}_{c}")
                        nc.vector.tensor_mul(xm[:], xx[:, c, cs],
                                             msk_all[:, cs])
                        nc.vector.tensor_reduce(
                            Ws[:, c, 2 * ti:2 * ti + nwt, :],
                            xm[:].rearrange("p (w n) -> p w n", n=N),
                            mybir.AxisListType.X, OP.add)
                        nc.vector.tensor_add(xm[:], xm[:], neg_all[:, cs])
                        nc.vector.tensor_reduce(
                            Wm[:, c, 2 * ti:2 * ti + nwt, :],
                            xm[:].rearrange("p (w n) -> p w n", n=N),
                            mybir.AxisListType.X, OP.max)
                col0 += tl

        # ===== PHASE 3: collectives (kicked first) + spatial path =====
        with tc.tile_pool(name="p3a", bufs=1) as p3a:
          with tc.tile_pool(name="w3", bufs=1) as wp3, \
               tc.tile_pool(name="p3", bufs=2) as p3, \
               tc.tile_pool(name="p3f", bufs=2) as p3f, \
               tc.tile_pool(name="zps", bufs=6, space="PSUM") as zps, \
               tc.tile_pool(name="sps", bufs=2, space="PSUM") as sps:
              cw1wTs = load_w(wp3, [128, 24, 4 * C // NCORE],
                              cw1wTs_d[:].rearrange("(c p) m -> p c m", p=128),
                              "cw1")
              cw2wTs = load_w(wp3, [128, 24, 2 * C // NCORE],
                              cw2wTs_d[:].rearrange("(c p) m -> p c m", p=128),
                              "cw2")
              sw1wT = load_w(wp3, [128, 2 * NCH, C], sw1wT_d[:].rearrange(
                  "(c p) m -> p c m", p=128), "sw1")
              sw2wT = cp.tile([128, NCH, 2], BF)
              nc.sync.dma_start(sw2wT[:], r6(sw2wT_d[:]))
              ident128 = p3a.tile([128, 128], F32)
              nc.vector.tensor_copy(ident128[:], identf[:])
              s01_all = p3a.tile([2, NTOK], F32)
              s1_row = p3a.tile([1, NTOK], F32)

              # ---- combine windows -> images, kick AllReduces ASAP ----
              imb = p3a.tile([128, 4, NW], F32)
              inb = p3a.tile([128, 4, NW], F32)
              for i in range(4):
                  r = stg.tile([1, NW], F32, tag="imrow")
                  nc.sync.dma_start(r[:], imgmask_d[i:i + 1, :])
                  nc.gpsimd.partition_broadcast(imb[:, i, :], r[:])
                  r2 = stg.tile([1, NW], F32, tag="imrow")
                  nc.sync.dma_start(r2[:], imgneg_d[i:i + 1, :])
                  nc.gpsimd.partition_broadcast(inb[:, i, :], r2[:])
              stat_s = p3a.tile([128, NCH, 2, 4], F32)
              stat_m = p3a.tile([128, NCH, 2, 4], F32)
              for k, Wt in ((0, W_s1), (1, W_s2)):
                  for i in range(4):
                      t = p3.tile([128, NCH, NW], F32, tag="cmb",
                                  name=f"cmb_{k}_{i}")
                      nc.vector.tensor_mul(t[:], Wt[:, :, 0:NW, 0],
                                           imb[:, i, None, :].broadcast_to(
                                               [128, NCH, NW]))
                      nc.vector.tensor_reduce(stat_s[:, :, k, i:i + 1], t[:],
                                              mybir.AxisListType.X, OP.add)
              for k, Wt in ((0, W_m1), (1, W_m2)):
                  for i in range(4):
                      t = p3.tile([128, NCH, NW], F32, tag="cmb",
                                  name=f"cmbm_{k}_{i}")
                      nc.vector.tensor_mul(t[:], Wt[:, :, 0:NW, 0],
                                           imb[:, i, None, :].broadcast_to(
                                               [128, NCH, NW]))
                      nc.vector.tensor_add(t[:], t[:],
                                           inb[:, i, None, :].broadcast_to(
                                               [128, NCH, NW]))
                      nc.vector.tensor_reduce(stat_m[:, :, k, i:i + 1], t[:],
                                              mybir.AxisListType.X, OP.max)
              nc.sync.dma_start(csum_in[:],
                                stat_s[:].rearrange("p a b c -> p (a b c)"))
              nc.sync.dma_start(cmax_in[:],
                                stat_m[:].rearrange("p a b c -> p (a b c)"))
              nc.gpsimd.collective_compute("AllReduce", OP.add,
                                           replica_groups=[core_ids],
                                           ins=[csum_in[:]], outs=[csum_out[:]])
              nc.gpsimd.collective_compute("AllReduce", OP.max,
                                           replica_groups=[core_ids],
                                           ins=[cmax_in[:]], outs=[cmax_out[:]])

              # ---- spatial sw path (PE work overlapping collectives) ----
              col0 = 0
              for ti, tl in enumerate(P34_TILES):
                  cs = slice(col0, col0 + tl)
                  zpsl = [zps.tile([128, tl], F32, tag="zp",
                                   name=f"zp_{col0}_{m}") for m in range(NCH)]
                  for mc in range(NCH):
                      for kc in range(2 * NCH):
                          rhs = (rgb_all[:, kc, cs] if kc < NCH
                                 else x_all[:, kc - NCH, cs])
                          nc.tensor.matmul(zpsl[mc][:],
                                           sw1wT[:, kc, mc * 128:(mc + 1) * 128],
                                           rhs, start=(kc == 0),
                                           stop=(kc == 2 * NCH - 1))
                  z_r = p3f.tile([128, NCH, tl], BF, tag="z_r", name=f"zr_{col0}")
                  for mc in range(NCH):
                      nc.scalar.activation(z_r[:, mc, :], zpsl[mc][:], AF.Relu,
                                           bias=sw1b_t[:, mc:mc + 1], scale=1.0)
                  sps_t = sps.tile([2, tl], F32, tag="sp", name=f"sp_{col0}")
                  for kc in range(NCH):
                      nc.tensor.matmul(sps_t[:], sw2wT[:, kc, :], z_r[:, kc, :],
                                       start=(kc == 0), stop=(kc == NCH - 1))
                  nc.vector.tensor_scalar_add(s01_all[:, cs], sps_t[:],
                                              sw2b_t[:])
                  col0 += tl
              # sigmoid(s)/2, split row 1 to partition 0
              nc.scalar.activation(s01_all[:], s01_all[:], AF.Sigmoid)
              nc.vector.tensor_scalar_mul(s01_all[:], s01_all[:], 0.5)
              nc.sync.dma_start(s1_row[:], s01_all[1:2, :])

              # channel MLP (sharded): ycat rhs [128, 24, 4]
              ycat_f = p3a.tile([128, 24, 4], F32)
              cso4 = csum_out[:].rearrange("p (c k i) -> p c k i", k=2, i=4)
              cmo4 = cmax_out[:].rearrange("p (c k i) -> p c k i", k=2, i=4)
              nc.sync.dma_start(ycat_f[:, 0:6, :], cso4[:, :, 0, :])
              nc.sync.dma_start(ycat_f[:, 6:12, :], cso4[:, :, 1, :])
              nc.sync.dma_start(ycat_f[:, 12:18, :], cmo4[:, :, 0, :])
              nc.sync.dma_start(ycat_f[:, 18:24, :], cmo4[:, :, 1, :])
              ycat_r = p3a.tile([128, 24, 4], BF)
              nc.vector.tensor_copy(ycat_r[:], ycat_f[:])
              z1sb = p3a.tile([128, 3, 4], BF)
              for mc in range(3):
                  ps = sps.tile([128, 4], F32, tag="sp", name=f"z1ps_{mc}")
                  for kc in range(24):
                      nc.tensor.matmul(ps[:], cw1wTs[:, kc, mc * 128:(mc + 1) * 128],
                                       ycat_r[:, kc, :], start=(kc == 0),
                                       stop=(kc == 23))
                  nc.scalar.activation(z1sb[:, mc, :], ps[:], AF.Relu,
                                       bias=cw1bs_t[:, mc:mc + 1], scale=1.0)
              z1f32 = p3a.tile([128, 3, 4], F32)
              nc.vector.tensor_copy(z1f32[:], z1sb[:])
              nc.sync.dma_start(z1_in[:].rearrange("(m p) f -> p m f", p=128),
                                z1f32[:])
              nc.gpsimd.collective_compute("AllGather", OP.bypass,
                                           replica_groups=[core_ids],
                                           ins=[z1_in[:]], outs=[z1_out[:]])
              z1f = p3a.tile([128, 24, 4], F32)
              nc.sync.dma_start(z1f[:],
                                z1_out[:].rearrange("(c p) f -> p c f", p=128))
              z1r = p3a.tile([128, 24, 4], BF)
              nc.vector.tensor_copy(z1r[:], z1f[:])
              z2sb = p3a.tile([128, 2, 4], F32)
              nc.any.memset(z2sb[:], 0.0)
              for mc, msz in ((0, 128), (1, 64)):
                  ps = sps.tile([128, 4], F32, tag="sp", name=f"z2ps_{mc}")
                  for kc in range(24):
                      nc.tensor.matmul(ps[0:msz, :],
                                       cw2wTs[:, kc, mc * 128:mc * 128 + msz],
                                       z1r[:, kc, :], start=(kc == 0),
                                       stop=(kc == 23))
                  nc.vector.tensor_scalar_add(z2sb[0:msz, mc, :], ps[0:msz, :],
                                              cw2bs_t[0:msz, mc:mc + 1])
              nc.sync.dma_start(z2_in[0:128, :], z2sb[:, 0, :])
              nc.sync.dma_start(z2_in[128:192, :], z2sb[0:64, 1, :])
              nc.gpsimd.collective_compute("AllGather", OP.bypass,
                                           replica_groups=[core_ids],
                                           ins=[z2_in[:]], outs=[z2_out[:]])
              y_f = p3a.tile([128, 12, 4], F32)
              nc.sync.dma_start(y_f[:],
                                z2_out[:].rearrange("(c p) f -> p c f", p=128))
              nc.scalar.activation(y_f[:], y_f[:], AF.Sigmoid)
              nc.vector.tensor_scalar_mul(y_f[:], y_f[:], 0.5)
              # transpose per chunk to [4, 128] bf16 for P4 matmuls
              cw0T = p3a.tile([4, NCH, 128], BF)   # y[:, :C]  (scales x1 -> out2)
              cw1T = p3a.tile([4, NCH, 128], BF)   # y[:, C:]  (scales x2 -> out1)
              for c in range(NCH):
                  for (dstt, src) in ((cw0T, y_f[:, c, :]),
                                      (cw1T, y_f[:, 6 + c, :])):
                      pstt = sps.tile([4, 128], F32, tag="sp",
                                      name=f"ct_{c}_{dstt.name}")
                      nc.tensor.transpose(pstt[:], src, ident128[:])
                      nc.scalar.copy(dstt[:, c, :], pstt[:])
              # rows of sw path as bf16 for rank-1 fold into P4 psums
              s0_bf = p3a.tile([1, NTOK], BF)
              nc.vector.tensor_copy(s0_bf[:], s01_all[0:1, :])
              s1_bf = p3a.tile([1, NTOK], BF)
              nc.vector.tensor_copy(s1_bf[:], s1_row[:])

          # ============ PHASE 4: final combine ============
          with tc.tile_pool(name="p4", bufs=2) as p4, \
               tc.tile_pool(name="cwp", bufs=4, space="PSUM") as cwp:
              imgsel_r = p4.tile([4, NTOK], BF, tag="imsr")
              nc.sync.dma_start(imgsel_r[:], imgsel_d[:])
              col0 = 0
              for tl in P34_TILES:
                  cs = slice(col0, col0 + tl)
                  o1 = p4.tile([128, NCH, tl], BF, tag="o1", name=f"o1_{col0}")
                  o2 = p4.tile([128, NCH, tl], BF, tag="o2", name=f"o2_{col0}")
                  for c in range(NCH):
                      pc0 = cwp.tile([128, tl], F32, tag="cw",
                                     name=f"c0_{col0}_{c}")
                      nc.tensor.matmul(pc0[:], cw0T[:, c, :], imgsel_r[:, cs],
                                       start=True, stop=False)
                      nc.tensor.matmul(pc0[:], ones_row[:], s0_bf[:, cs],
                                       start=False, stop=True)
                      pc1 = cwp.tile([128, tl], F32, tag="cw",
                                     name=f"c1_{col0}_{c}")
                      nc.tensor.matmul(pc1[:], cw1T[:, c, :], imgsel_r[:, cs],
                                       start=True, stop=False)
                      nc.tensor.matmul(pc1[:], ones_row[:], s1_bf[:, cs],
                                       start=False, stop=True)
                      t0 = p4.tile([128, tl], BF, tag="t0", name=f"t0_{col0}_{c}")
                      nc.vector.tensor_mul(t0[:], pc1[:], x_all[:, c, cs])
                      nc.vector.tensor_add(o1[:, c, :], rgb_all[:, c, cs], t0[:])
                      t1 = p4.tile([128, tl], BF, tag="t1", name=f"t1_{col0}_{c}")
                      nc.vector.tensor_mul(t1[:], pc0[:], rgb_all[:, c, cs])
                      nc.vector.tensor_add(o2[:, c, :], x_all[:, c, cs], t1[:])
                  nc.sync.dma_start(r6(out1_d[:])[:, :, cs], o1[:])
                  nc.sync.dma_start(r6(out2_d[:])[:, :, cs], o2[:])
                  col0 += tl

    nc.compile()
    return nc


def _windowize(x):
    # [B, 64, 64, C] -> [104, 196, C] padded windows
    Bp = np.zeros((B, 70, 70, C), x.dtype)
    Bp[:, :64, :64, :] = x
    w = Bp.reshape(B, GRID, WIN, GRID, WIN, C).transpose(0, 1, 3, 2, 4, 5)
    w = w.reshape(NWIN_TOT, N, C)
    out = np.zeros((NCORE * NW, N, C), x.dtype)
    out[:NWIN_TOT] = w
    return out


def _unwindowize(perwin):
    # [104, 196, C] -> [B, 64, 64, C]
    w = perwin[:NWIN_TOT].reshape(B, GRID, GRID, WIN, WIN, C)
    w = w.transpose(0, 1, 3, 2, 4, 5).reshape(B, 70, 70, C)
    return np.ascontiguousarray(w[:, :64, :64, :])


def kernel(rgb_embedding, x_embedding, norm1_w, norm1_b, qkv_w, qkv_b,
           rel_pos_h, rel_pos_w, proj_w, proj_b, norm2_w, norm2_b,
           fc1_w, fc1_b, fc2_w, fc2_b, cw1_w, cw1_b, cw2_w, cw2_b,
           sw1_w, sw1_b, sw2_w, sw2_b):
    if "nc" not in _CACHE:
        _CACHE["nc"] = _build()
    nc = _CACHE["nc"]

    f32 = lambda a: np.ascontiguousarray(a, dtype=np.float32)
    bf = lambda a: np.ascontiguousarray(np.asarray(a, dtype=np.float32)
                                        .astype(NPBF))
    f8w = lambda a: np.ascontiguousarray(
        np.clip(np.asarray(a, np.float32), -240, 240)
        .astype(ml_dtypes.float8_e4m3))
    xw = _windowize(f32(x_embedding))        # [104, 196, C]
    rw = _windowize(f32(rgb_embedding))
    vm = np.zeros((NCORE * NW, N), np.float32)
    vh = np.minimum(np.maximum(64 - np.arange(GRID) * WIN, 0), WIN)
    wm = np.zeros((GRID, GRID, WIN, WIN), np.float32)
    for a in range(GRID):
        for b in range(GRID):
            wm[a, b, :vh[a], :vh[b]] = 1.0
    vm[:NWIN_TOT] = np.tile(wm.reshape(GRID * GRID, N), (B, 1))
    win_img = np.full(NCORE * NW, -1, np.int64)
    win_img[:NWIN_TOT] = np.arange(NWIN_TOT) // (GRID * GRID)

    qkb = f32(qkv_b[:2 * C]).copy()
    qkb[:C] *= SCALE
    tab = np.concatenate([f32(rel_pos_h)[::-1], f32(rel_pos_w)[::-1]], axis=0)
    tab = np.ascontiguousarray(tab.T) * (1.0 / SCALE)
    cw1s = f32(cw1_w).T.copy()              # [4C(k), 4C(m)]
    cw1s[:2 * C, :] *= 1.0 / (HH * WW)      # fold avg divisor
    shared = dict(
        qkwT=bf(f32(qkv_w)[:2 * C].T), qkb=qkb,
        vwT=bf(f32(qkv_w)[2 * C:].T), vb=f32(qkv_b[2 * C:]).copy(),
        projwT=bf(f32(proj_w).T), projb=f32(proj_b),
        n1w=f32(norm1_w), n1b=f32(norm1_b), n2w=f32(norm2_w), n2b=f32(norm2_b),
        tab=bf(tab),
        fc1w8=f8w(f32(fc1_w).T.reshape(3, 2, 128, DFF)
                  .transpose(2, 0, 1, 3)), fc1b=f32(fc1_b),
        fc2wT=bf(f32(fc2_w).T), fc2b=f32(fc2_b),
        sw1wT=bf(f32(sw1_w).T), sw1b=f32(sw1_b),
        sw2wT=bf(f32(sw2_w).T), sw2b=f32(sw2_b),
    )
    cw2s = np.ascontiguousarray(f32(cw2_w).T)  # [4C, 2C]
    in_maps = []
    for c in range(NCORE):
        sl = slice(c * NW, (c + 1) * NW)
        xT = bf(xw[sl].reshape(NTOK, C).T)
        rT = bf(rw[sl].reshape(NTOK, C).T)
        mrow = vm[sl].reshape(1, NTOK).copy()
        imgm = np.zeros((4, NW), np.float32)
        imsel = np.zeros((4, NTOK), np.float32)
        for wloc in range(NW):
            im = win_img[c * NW + wloc]
            if im >= 0:
                imgm[im, wloc] = 1.0
                imsel[im, wloc * N:(wloc + 1) * N] = 1.0
        m = dict(shared)
        m.update(
            xT=xT, rgbT=rT, mask=bf(mrow),
            imgmask=imgm, imgneg=(imgm - 1.0) * 1e30, imgsel=bf(imsel),
            cw1wTs=bf(cw1s[:, c * 384:(c + 1) * 384]),
            cw1bs=f32(cw1_b[c * 384:(c + 1) * 384]).copy(),
            cw2wTs=bf(cw2s[:, c * 192:(c + 1) * 192]),
            cw2bs=f32(cw2_b[c * 192:(c + 1) * 192]).copy(),
        )
        in_maps.append(m)

    trace = bool(os.environ.get("KERNEL_TRACE"))
    res = run_bass_kernel_spmd(nc, in_maps, list(range(NCORE)), trace=trace)
    if trace:
        _CACHE["exec_time_ns"] = res.exec_time_ns
    o1 = np.zeros((NCORE * NW, N, C), np.float32)
    o2 = np.zeros((NCORE * NW, N, C), np.float32)
    for c in range(NCORE):
        sl = slice(c * NW, (c + 1) * NW)
        o1[sl] = res.results[c]["out1T"].astype(np.float32).T.reshape(NW, N, C)
        o2[sl] = res.results[c]["out2T"].astype(np.float32).T.reshape(NW, N, C)
    rgb_out = _unwindowize(o1)
    x_out = _unwindowize(o2)
    return rgb_out, x_out


# revision 41
# speedup vs baseline: 1.0026x; 1.0026x over previous
"""SAM-block (windowed attention + MLP) + FRM fusion on 8 TRN2 NeuronCores.

v2: bf16 datapath (fp32 PSUM accumulation), SBUF-resident intermediates,
batched rel-pos gather DMAs, per-window score matmuls, bf16 outputs.

Self-contained: shards the 100 attention windows over 8 cores (13/core),
runs one SPMD Bass program via run_bass_kernel_spmd, reassembles on host.
Device layout: [C(partitions), tokens(free)] for projections; attention
blocks operate on window pairs (392 tokens) with tokens on partitions.
"""
import os
import numpy as np
import ml_dtypes
import concourse.bass as bass
import concourse.bacc as bacc
import concourse.mybir as mybir
from concourse import tile
from concourse.masks import make_identity
from concourse.bass_utils import run_bass_kernel_spmd

F32 = mybir.dt.float32
BF = mybir.dt.bfloat16
F8 = mybir.dt.float8e4
AF = mybir.ActivationFunctionType
OP = mybir.AluOpType
NPBF = ml_dtypes.bfloat16

# problem constants
B, HH, WW, C = 4, 64, 64, 768
WIN, NH, HD = 14, 12, 64
S = WIN
N = S * S                  # 196 tokens / window
GRID = 5                   # 5x5 windows per image (64 -> 70 padded)
NWIN_TOT = B * GRID * GRID  # 100
NCORE = 8
NW = 13                    # windows per core (104 slots, 4 dummy)
NTOK = NW * N              # 2548
NT = 2 * N                 # 392 = pair block
NCH = C // 128             # 6
DFF = 4 * C
SCALE = HD ** -0.5
EPS = 1e-6
# 6 pair blocks + 1 single-window block (window 12)
BLOCKS = [(i * NT, 2) for i in range(6)] + [(6 * NT, 1)]
P2_TILES = [392] * 6 + [196]
P34_TILES = [392] * 6 + [196]

_CACHE = {}


def _build():
    nc = bacc.Bacc("TRN2", target_bir_lowering=False, debug=False)
    dt_in = {}

    def din(name, shape, dt=BF):
        dt_in[name] = nc.dram_tensor(name, shape, dt, kind="ExternalInput")
        return dt_in[name]

    xT_d = din("xT", [C, NTOK])
    rgbT_d = din("rgbT", [C, NTOK])
    mask_d = din("mask", [1, NTOK], BF)
    imgmask_d = din("imgmask", [4, NW], F32)
    imgneg_d = din("imgneg", [4, NW], F32)
    imgsel_d = din("imgsel", [4, NTOK], BF)
    qkwT_d = din("qkwT", [C, 2 * C])
    qkb_d = din("qkb", [2 * C], F32)
    vwT_d = din("vwT", [C, C])
    vb_d = din("vb", [C], F32)
    projwT_d = din("projwT", [C, C])
    projb_d = din("projb", [C], F32)
    n1w_d = din("n1w", [C], F32)
    n1b_d = din("n1b", [C], F32)
    n2w_d = din("n2w", [C], F32)
    n2b_d = din("n2b", [C], F32)
    tab_d = din("tab", [HD, 54])
    fc1w8_d = din("fc1w8", [128, 3, 2, DFF], F8)
    fc1b_d = din("fc1b", [DFF], F32)
    fc2wT_d = din("fc2wT", [DFF, C])
    fc2b_d = din("fc2b", [C], F32)
    sw1wT_d = din("sw1wT", [2 * C, C])
    sw1b_d = din("sw1b", [C], F32)
    sw2wT_d = din("sw2wT", [C, 2])
    sw2b_d = din("sw2b", [2], F32)
    cw1wTs_d = din("cw1wTs", [4 * C, 4 * C // NCORE])
    cw1bs_d = din("cw1bs", [4 * C // NCORE], F32)
    cw2wTs_d = din("cw2wTs", [4 * C, 2 * C // NCORE])
    cw2bs_d = din("cw2bs", [2 * C // NCORE], F32)
    out1_d = nc.dram_tensor("out1T", [C, NTOK], BF, kind="ExternalOutput")
    out2_d = nc.dram_tensor("out2T", [C, NTOK], BF, kind="ExternalOutput")

    core_ids = list(range(NCORE))
    r6 = lambda ap: ap.rearrange("(c p) n -> p c n", p=128)
    rcol = lambda ap: ap.rearrange("(c p) -> p c", p=128)

    with tile.TileContext(nc) as tc:
      with tc.tile_pool(name="dram", bufs=1, space="DRAM") as dramp, \
           tc.tile_pool(name="fbp", bufs=2, space="DRAM") as fbp, \
           tc.tile_pool(name="cst", bufs=1) as cp, \
           tc.tile_pool(name="res", bufs=1) as resp, \
           tc.tile_pool(name="stg", bufs=2) as stg:
        # ---------- DRAM scratch (collectives only) ----------
        csum_in = dramp.tile([128, 48], F32)
        csum_out = dramp.tile([128, 48], F32, addr_space="Shared")
        cmax_in = dramp.tile([128, 48], F32)
        cmax_out = dramp.tile([128, 48], F32, addr_space="Shared")
        z1_in = dramp.tile([4 * C // NCORE, 4], F32)
        z1_out = dramp.tile([4 * C, 4], F32, addr_space="Shared")
        z2_in = dramp.tile([2 * C // NCORE, 4], F32)
        z2_out = dramp.tile([2 * C, 4], F32, addr_space="Shared")

        # ---------- SBUF-resident activations ----------
        # x_all holds x -> (in-place) x2a = x+attn -> (in-place) x2 = x2a+mlp
        x_all = resp.tile([128, NCH, NTOK], BF)
        nc.sync.dma_start(x_all[:], r6(xT_d[:]))
        rgb_all = resp.tile([128, NCH, NTOK], BF)     # rgb input

        # ---------- persistent constants ----------
        identf = stg.tile([128, 128], F32, tag="st1")
        make_identity(nc, identf)
        ident98 = cp.tile([98, 98], BF)
        nc.vector.tensor_copy(ident98[:], identf[0:98, 0:98])

        def load_rows(src, n=C):
            t = cp.tile([128, n // 128], F32, name="rows_" + src.tensor.name)
            nc.sync.dma_start(t[:], rcol(src))
            return t

        qkb_t = load_rows(qkb_d[:], 2 * C)
        projb_t = load_rows(projb_d[:])
        n1w_t = load_rows(n1w_d[:])
        n1b_t = load_rows(n1b_d[:])
        n2w_t = load_rows(n2w_d[:])
        n2b_t = load_rows(n2b_d[:])
        fc1b_t = load_rows(fc1b_d[:], DFF)
        fc2b_t = load_rows(fc2b_d[:])
        sw1b_t = load_rows(sw1b_d[:])
        sw2b_t = cp.tile([2, 1], F32)
        nc.sync.dma_start(sw2b_t[:, 0], sw2b_d[:])
        cw1bs_t = load_rows(cw1bs_d[:], 4 * C // NCORE)
        cw2bs_t = cp.tile([128, 2], F32)
        nc.any.memset(cw2bs_t[:], 0.0)
        nc.sync.dma_start(cw2bs_t[0:128, 0], cw2bs_d[0:128])
        nc.sync.dma_start(cw2bs_t[0:64, 1], cw2bs_d[128:192])
        vb_t = load_rows(vb_d[:])
        tab2 = cp.tile([128, 2, 54], BF)
        nc.any.memset(tab2[:], 0.0)
        nc.sync.dma_start(tab2[0:64, 0, :], tab_d[:])
        nc.sync.dma_start(tab2[64:128, 1, :], tab_d[:])
        ones_f = stg.tile([128, 1], F32, tag="st1")
        nc.any.memset(ones_f[:], 1.0)
        ones_r = cp.tile([128, 1], BF)
        nc.vector.tensor_copy(ones_r[:], ones_f[:])
        ones_row = cp.tile([1, 128], BF)
        nc.any.memset(ones_row[:], 1.0)
        mask_bf = cp.tile([1, NTOK], BF)
        nc.sync.dma_start(mask_bf[:], mask_d[:])

        def load_w(pool_, shape3, src_ap, nm, dt=BF):
            # weights load directly (no staging/round)
            r = pool_.tile(shape3, dt, name="w_" + nm)
            nc.sync.dma_start(r[:], src_ap)
            return r

        # ==================== PHASE 1: attention ====================
        with tc.tile_pool(name="w1", bufs=1) as wp1, \
             tc.tile_pool(name="p1", bufs=1) as p1, \
             tc.tile_pool(name="p1b", bufs=1) as p1b, \
             tc.tile_pool(name="p1r", bufs=1) as p1r, \
             tc.tile_pool(name="p1s", bufs=2) as p1s, \
             tc.tile_pool(name="p1f", bufs=2) as p1f, \
             tc.tile_pool(name="ln", bufs=2, space="PSUM") as psln, \
             tc.tile_pool(name="gen", bufs=3, space="PSUM") as psg, \
             tc.tile_pool(name="pst", bufs=1, space="PSUM") as pst, \
             tc.tile_pool(name="psa", bufs=2, space="PSUM") as psa:
            qkwT = load_w(wp1, [128, NCH, 2 * C], r6(qkwT_d[:]), "qk")
            vwT = load_w(wp1, [128, NCH, C], r6(vwT_d[:]), "v")
            projwT = load_w(wp1, [128, NCH, C], r6(projwT_d[:]), "pj")

            kzAB = [p1.tile([128, NH, NT], BF, tag=f"kz{i}", name=f"kz{i}")
                    for i in range(2)]
            nc.any.memset(kzAB[0][:], 0.0)
            nc.any.memset(kzAB[1][:], 0.0)

            for blk, (c0, nw) in enumerate(BLOCKS):
                NTb = nw * N
                cols = slice(c0, c0 + NTb)
                kzt = kzAB[blk % 2]

                # ---- LN1 stats ----
                s1 = psln.tile([1, NT], F32, tag="lnp")
                s2 = psln.tile([1, NT], F32, tag="lnp")
                for c in range(NCH):
                    nc.tensor.matmul(s1[:, 0:NTb], ones_r[:], x_all[:, c, cols],
                                     start=(c == 0), stop=(c == NCH - 1))
                for c in range(NCH):
                    xsqc = p1f.tile([128, NT], BF, tag="xsqc")
                    nc.vector.tensor_mul(xsqc[:, 0:NTb], x_all[:, c, cols],
                                         x_all[:, c, cols])
                    nc.tensor.matmul(s2[:, 0:NTb], ones_r[:], xsqc[:, 0:NTb],
                                     start=(c == 0), stop=(c == NCH - 1))
                mrow_t = p1r.tile([1, NT], F32, tag="r1")
                nc.scalar.activation(mrow_t[:, 0:NTb], s1[:, 0:NTb], AF.Copy,
                                     scale=1.0 / C)
                var = p1r.tile([1, NT], F32, tag="r2")
                nc.vector.tensor_mul(var[:, 0:NTb], mrow_t[:, 0:NTb],
                                     mrow_t[:, 0:NTb])
                e2 = p1r.tile([1, NT], F32, tag="r3")
                nc.scalar.activation(e2[:, 0:NTb], s2[:, 0:NTb], AF.Copy,
                                     scale=1.0 / C)
                nc.vector.tensor_sub(var[:, 0:NTb], e2[:, 0:NTb], var[:, 0:NTb])
                nc.vector.tensor_scalar_add(var[:, 0:NTb], var[:, 0:NTb], EPS)
                sd = p1r.tile([1, NT], F32, tag="r4")
                nc.scalar.activation(sd[:, 0:NTb], var[:, 0:NTb], AF.Sqrt)
                inv = p1r.tile([1, NT], F32, tag="r5")
                nc.vector.reciprocal(inv[:, 0:NTb], sd[:, 0:NTb])
                mrow_b = p1r.tile([1, NT], BF, tag="r6")
                nc.vector.tensor_copy(mrow_b[:, 0:NTb], mrow_t[:, 0:NTb])
                # fold the window-validity mask into inv (norm1_b == 0)
                inv_bf = p1r.tile([1, NT], BF, tag="r7")
                nc.vector.tensor_mul(inv_bf[:, 0:NTb], inv[:, 0:NTb],
                                     mask_bf[:, cols])
                m_b = p1f.tile([128, NT], BF, tag="m_b")
                nc.gpsimd.partition_broadcast(m_b[:, 0:NTb], mrow_b[:, 0:NTb])
                inv_b = p1f.tile([128, NT], BF, tag="inv_b")
                nc.gpsimd.partition_broadcast(inv_b[:, 0:NTb], inv_bf[:, 0:NTb])
                lnx = p1b.tile([128, NCH, NT], BF, tag="lnx")
                for c in range(NCH):
                    u = p1s.tile([128, NT], BF, tag="u")
                    nc.vector.tensor_sub(u[:, 0:NTb], x_all[:, c, cols],
                                         m_b[:, 0:NTb])
                    nc.vector.tensor_mul(u[:, 0:NTb], u[:, 0:NTb],
                                         inv_b[:, 0:NTb])
                    nc.scalar.activation(lnx[:, c, 0:NTb], u[:, 0:NTb],
                                         AF.Identity, bias=n1b_t[:, c:c + 1],
                                         scale=n1w_t[:, c:c + 1])

                # ---- qk projection ----
                qT = p1b.tile([128, NCH, NT], BF, tag="qT")
                for mc in range(12):
                    ps = psg.tile([128, NT], F32, tag="gen")
                    for kc in range(NCH):
                        nc.tensor.matmul(ps[:, 0:NTb],
                                         qkwT[:, kc, mc * 128:(mc + 1) * 128],
                                         lnx[:, kc, 0:NTb], start=(kc == 0),
                                         stop=(kc == NCH - 1))
                    if mc < NCH:
                        nc.scalar.activation(qT[:, mc, 0:NTb], ps[:, 0:NTb],
                                             AF.Identity,
                                             bias=qkb_t[:, mc:mc + 1],
                                             scale=SCALE)
                    else:
                        j = mc - NCH
                        nc.scalar.activation(kzt[0:64, 2 * j, 0:NTb],
                                             ps[0:64, 0:NTb], AF.Identity,
                                             bias=qkb_t[0:64, mc:mc + 1],
                                             scale=1.0)
                        nc.scalar.activation(kzt[64:128, 2 * j + 1, 0:NTb],
                                             ps[64:128, 0:NTb], AF.Identity,
                                             bias=qkb_t[64:128, mc:mc + 1],
                                             scale=1.0)

                # ---- f features (rel-pos), batched ----
                # layout [p, qc, w, h, t] so (w,h) is one contiguous dim
                fall = p1b.tile([98, 2, 2, NH, 54], BF, tag="fall",
                                name=f"fall_{blk}")
                for qc in range(2):
                    for w in range(nw):
                        ps1 = psg.tile([98, 9 * 54], F32, tag="gen")
                        ps2 = psg.tile([98, 3 * 54], F32, tag="gen")
                        for h in range(NH):
                            dst = ps1[:, h * 54:h * 54 + 54] if h < 9 else \
                                ps2[:, (h - 9) * 54:(h - 9) * 54 + 54]
                            lhsT = qT[:, h // 2, w * N + qc * 98: w * N + qc * 98 + 98]
                            nc.tensor.matmul(dst, lhsT, tab2[:, h % 2, :],
                                             start=True, stop=True)
                        nc.scalar.copy(fall[:, qc, w, 0:9, :],
                                       ps1[:].rearrange("p (h t) -> p h t", t=54))
                        nc.scalar.copy(fall[:, qc, w, 9:12, :],
                                       ps2[:].rearrange("p (h t) -> p h t", t=54))
                fd = fbp.tile([98, 2, 2, NH, 54], BF, tag="fb", name=f"fd_{blk}")
                nc.sync.dma_start(fd[:], fall[:])
                gh = p1f.tile([98, 2, 2, NH, 14], BF, tag="gh", name=f"gh_{blk}")
                gw = p1f.tile([98, 2, 2, NH, 14], BF, tag="gw", name=f"gw_{blk}")
                for qc in range(2):
                    for g in range(7):
                        ih = qc * 7 + g
                        nc.sync.dma_start(
                            gh[g * 14:(g + 1) * 14, qc, :, :, :],
                            fall[g * 14:(g + 1) * 14, qc, :, :,
                                 13 - ih:27 - ih])
                        src_ap = bass.AP(fd[:].tensor,
                                         (g * 14) * 2592 + qc * 1296 + 40,
                                         [[2591, 14], [54, 2 * NH], [1, 14]])
                        nc.sync.dma_start(
                            gw[g * 14:(g + 1) * 14, qc, :, :, :].rearrange(
                                "p w h k -> p (w h) k"), src_ap)
                egh = p1f.tile([98, 2, 2, NH, 14], BF, tag="egh",
                               name=f"egh_{blk}")
                nc.scalar.activation(egh[:], gh[:], AF.Exp)
                egw = p1f.tile([98, 2, 2, NH, 14], BF, tag="egw",
                               name=f"egw_{blk}")
                nc.scalar.activation(egw[:], gw[:], AF.Exp)

                # ---- v (token-major pairs) ----
                v_pair = [p1b.tile([98, NH, 2, 64], BF, tag=f"vp{i}",
                                   name=f"vp{i}_{blk}") for i in range(2)]
                for c4 in range(2 * nw):
                    ps_a = psg.tile([98, 512], F32, tag="gen")
                    ps_b = psg.tile([98, 256], F32, tag="gen")
                    for kc in range(NCH):
                        nc.tensor.matmul(ps_a[:], lnx[:, kc, c4 * 98:(c4 + 1) * 98],
                                         vwT[:, kc, 0:512], start=(kc == 0),
                                         stop=(kc == NCH - 1))
                    for kc in range(NCH):
                        nc.tensor.matmul(ps_b[:], lnx[:, kc, c4 * 98:(c4 + 1) * 98],
                                         vwT[:, kc, 512:768], start=(kc == 0),
                                         stop=(kc == NCH - 1))
                    nc.scalar.copy(
                        v_pair[c4 % 2][:, 0:8, c4 // 2, :],
                        ps_a[:].rearrange("p (h d) -> p h d", d=64))
                    nc.scalar.copy(
                        v_pair[c4 % 2][:, 8:12, c4 // 2, :],
                        ps_b[:].rearrange("p (h d) -> p h d", d=64))

                # ---- scores + softmax + transpose + AV ----
                attn_outT = p1b.tile([128, NCH, NT], BF, tag="attn_outT")
                GH = 2
                for grp in range(NH // GH):
                    P_rg = {}
                    for w in range(nw):
                        for qc in range(2):
                            sco = p1f.tile([98, GH, S, S], BF, tag="sco")
                            ps = psg.tile([98, GH, S, S], F32, tag="gen")
                            lhsT = qT[:, grp,
                                      w * N + qc * 98: w * N + qc * 98 + 98]
                            for hh in range(GH):
                                h = grp * GH + hh
                                nc.tensor.matmul(ps[:, hh, :, :], lhsT,
                                                 kzt[:, h, w * N:(w + 1) * N],
                                                 start=True, stop=True)
                            nc.scalar.activation(sco[:], ps[:], AF.Exp)
                            nc.vector.tensor_mul(
                                sco[:], sco[:],
                                egh[:, qc, w, grp * GH:grp * GH + GH, :, None]
                                .broadcast_to([98, GH, S, S]))
                            nc.vector.tensor_mul(
                                sco[:], sco[:],
                                egw[:, qc, w, grp * GH:grp * GH + GH, None, :]
                                .broadcast_to([98, GH, S, S]))
                            z = p1s.tile([98, GH, 1, 1], F32, tag="z")
                            nc.vector.tensor_reduce(z[:], sco[:],
                                                    mybir.AxisListType.XY, OP.add)
                            nc.vector.reciprocal(z[:], z[:])
                            prg = p1f.tile([98, GH, S, S], BF, tag=f"P{w}{qc}",
                                           name=f"P{w}{qc}_{blk}_{grp}")
                            for hh in range(GH):
                                nc.vector.tensor_scalar_mul(
                                    prg[:, hh, :, :], sco[:, hh, :, :],
                                    z[:, hh, 0, :])
                            P_rg[(w, qc)] = prg
                    for hh in range(GH):
                        h = grp * GH + hh
                        PT_sb = []
                        for kc in range(2):
                            psT = pst.tile([98, 392], BF, tag="psT")
                            for j, (w, qc) in enumerate(
                                    [(w, qc) for w in range(nw)
                                     for qc in range(2)]):
                                nc.tensor.transpose(
                                    psT[:, j * 98:(j + 1) * 98],
                                    P_rg[(w, qc)][:, hh, 7 * kc:7 * kc + 7, :],
                                    ident98[:])
                            sb = p1s.tile([98, 392], BF, tag="PTsb")
                            nc.scalar.copy(sb[:, 0:nw * N], psT[:, 0:nw * N])
                            PT_sb.append(sb)
                        psA = psa.tile([128, 392], F32, tag="psAV")
                        for kc in range(2):
                            nc.tensor.matmul(psA[0:64 * nw, 0:NTb],
                                             v_pair[kc][:, h, 0:nw, :],
                                             PT_sb[kc][:, 0:NTb], start=(kc == 0),
                                             stop=(kc == 1))
                        nc.scalar.activation(
                            attn_outT[(h % 2) * 64:(h % 2) * 64 + 64, h // 2,
                                      0:196],
                            psA[0:64, 0:196], AF.Identity,
                            bias=vb_t[(h % 2) * 64:(h % 2) * 64 + 64,
                                      h // 2:h // 2 + 1], scale=1.0)
                        if nw == 2:
                            nc.vector.tensor_scalar_add(
                                attn_outT[(h % 2) * 64:(h % 2) * 64 + 64, h // 2,
                                          196:392],
                                psA[64:128, 196:392],
                                vb_t[(h % 2) * 64:(h % 2) * 64 + 64,
                                     h // 2:h // 2 + 1])

                # ---- proj + residual -> x2a (SBUF) ----
                for mc in range(NCH):
                    ps = psg.tile([128, NT], F32, tag="gen")
                    for kc in range(NCH):
                        nc.tensor.matmul(ps[:, 0:NTb],
                                         projwT[:, kc, mc * 128:(mc + 1) * 128],
                                         attn_outT[:, kc, 0:NTb], start=(kc == 0),
                                         stop=(kc == NCH - 1))
                    tmp = p1s.tile([128, NT], BF, tag="projtmp")
                    nc.scalar.activation(tmp[:, 0:NTb], ps[:, 0:NTb], AF.Identity,
                                         bias=projb_t[:, mc:mc + 1], scale=1.0)
                    nc.vector.tensor_add(x_all[:, mc, cols], tmp[:, 0:NTb],
                                         x_all[:, mc, cols])

        # ============ PHASE 2: LN2 stat pre-pass + fp8 MLP ============
        m2_row = resp.tile([1, NTOK], BF, name="m2_row")
        i2_row = resp.tile([1, NTOK], BF, name="i2_row")
        msk_all = resp.tile([128, NTOK], BF, name="msk_all")
        nc.gpsimd.partition_broadcast(msk_all[:], mask_bf[:])
        neg_all = resp.tile([128, NTOK], BF, name="neg_all")
        nc.vector.tensor_scalar_add(neg_all[:], msk_all[:], -1.0)
        nc.vector.tensor_scalar_mul(neg_all[:], neg_all[:], 1e30)
        W_s1 = resp.tile([128, NCH, 14, 1], F32, name="W_s1")
        W_s2 = resp.tile([128, NCH, 14, 1], F32, name="W_s2")
        W_m1 = resp.tile([128, NCH, 14, 1], F32, name="W_m1")
        W_m2 = resp.tile([128, NCH, 14, 1], F32, name="W_m2")
        with tc.tile_pool(name="pre2", bufs=2) as pre2, \
             tc.tile_pool(name="psp", bufs=2, space="PSUM") as psp:
            # prefetch rgb for stats fusion + phases 3/4
            nc.sync.dma_start(rgb_all[:], r6(rgbT_d[:]))
            col0 = 0
            for tl in P2_TILES:
                cs = slice(col0, col0 + tl)
                s1 = psp.tile([1, tl], F32, tag="s1", name=f"pps1_{col0}")
                s2 = psp.tile([1, tl], F32, tag="s2", name=f"pps2_{col0}")
                for c in range(NCH):
                    nc.tensor.matmul(s1[:], ones_r[:], x_all[:, c, cs],
                                     start=(c == 0), stop=(c == NCH - 1))
                for c in range(NCH):
                    xsqc = pre2.tile([128, tl], BF, tag="xsqc",
                                     name=f"xsq2_{col0}_{c}")
                    nc.vector.tensor_mul(xsqc[:], x_all[:, c, cs],
                                         x_all[:, c, cs])
                    nc.tensor.matmul(s2[:], ones_r[:], xsqc[:],
                                     start=(c == 0), stop=(c == NCH - 1))
                mrow = pre2.tile([1, tl], F32, tag="r1", name=f"m2m_{col0}")
                nc.scalar.activation(mrow[:], s1[:], AF.Copy, scale=1.0 / C)
                var = pre2.tile([1, tl], F32, tag="r2", name=f"m2v_{col0}")
                nc.vector.tensor_mul(var[:], mrow[:], mrow[:])
                e2 = pre2.tile([1, tl], F32, tag="r3", name=f"m2e_{col0}")
                nc.scalar.activation(e2[:], s2[:], AF.Copy, scale=1.0 / C)
                nc.vector.tensor_sub(var[:], e2[:], var[:])
                nc.vector.tensor_scalar_add(var[:], var[:], EPS)
                sd = pre2.tile([1, tl], F32, tag="r4", name=f"m2s_{col0}")
                nc.scalar.activation(sd[:], var[:], AF.Sqrt)
                inv = pre2.tile([1, tl], F32, tag="r5", name=f"m2i_{col0}")
                nc.vector.reciprocal(inv[:], sd[:])
                nc.vector.tensor_copy(m2_row[:, cs], mrow[:])
                nc.vector.tensor_copy(i2_row[:, cs], inv[:])
                col0 += tl
        with tc.tile_pool(name="w2", bufs=1) as wp2, \
             tc.tile_pool(name="p2", bufs=2) as p2, \
             tc.tile_pool(name="p2f", bufs=2) as p2f, \
             tc.tile_pool(name="ps1p", bufs=2, space="PSUM") as ps1p, \
             tc.tile_pool(name="ps2p", bufs=6, space="PSUM") as ps2p:
            fc1w8 = load_w(wp2, [128, 3, 2, DFF], fc1w8_d[:], "fc1", dt=F8)
            fc2wT = load_w(wp2, [128, DFF // 128, C], r6(fc2wT_d[:]), "fc2")
            DR = mybir.MatmulPerfMode.DoubleRow
            col0 = 0
            for ti, tl in enumerate(P2_TILES):
                nwt = tl // N
                cs = slice(col0, col0 + tl)
                m_b = p2f.tile([128, tl], BF, tag="m_b", name=f"mb_{col0}")
                nc.gpsimd.partition_broadcast(m_b[:], m2_row[:, cs])
                inv_b = p2f.tile([128, tl], BF, tag="inv_b", name=f"ib_{col0}")
                nc.gpsimd.partition_broadcast(inv_b[:], i2_row[:, cs])
                ln28 = p2.tile([128, 3, 2, tl], F8, tag="ln28", name=f"l2_{col0}")
                for c in range(NCH):
                    u = p2.tile([128, tl], BF, tag="u", name=f"u_{col0}_{c}")
                    nc.vector.tensor_sub(u[:], x_all[:, c, cs], m_b[:])
                    nc.vector.tensor_mul(u[:], u[:], inv_b[:])
                    nc.scalar.activation(ln28[:, c // 2, c % 2, :], u[:],
                                         AF.Identity, bias=n2b_t[:, c:c + 1],
                                         scale=n2w_t[:, c:c + 1])
                h_bf = p2.tile([128, 24, tl], BF, tag="h8", name=f"h8_{col0}")
                psum2 = [ps2p.tile([128, tl], F32, tag="ps2",
                                   name=f"ps2_{col0}_{m}") for m in range(NCH)]
                for kt2 in range(12):
                    for jj in range(2):
                        m24 = kt2 * 2 + jj
                        ps1 = ps1p.tile([128, tl], F32, tag="ps1",
                                        name=f"ps1_{col0}_{m24}")
                        for kt in range(3):
                            nc.tensor.matmul(
                                ps1[:], fc1w8[:, kt, :, m24 * 128:(m24 + 1) * 128],
                                ln28[:, kt, :, :], start=(kt == 0),
                                stop=(kt == 2), perf_mode=DR)
                        nc.scalar.activation(h_bf[:, m24, :], ps1[:], AF.Gelu,
                                             bias=fc1b_t[:, m24:m24 + 1],
                                             scale=1.0)
                        for mc in range(NCH):
                            nc.tensor.matmul(
                                psum2[mc][:],
                                fc2wT[:, m24, mc * 128:(mc + 1) * 128],
                                h_bf[:, m24, :], start=(m24 == 0),
                                stop=(m24 == 23))
                for mc in range(NCH):
                    tmp = p2.tile([128, tl], BF, tag="fct", name=f"fct_{col0}_{mc}")
                    nc.scalar.activation(tmp[:], psum2[mc][:], AF.Identity,
                                         bias=fc2b_t[:, mc:mc + 1], scale=1.0)
                    nc.vector.tensor_add(x_all[:, mc, cs], tmp[:],
                                         x_all[:, mc, cs])
                # FRM per-window stats fused here (x1 = rgb, x2 = x_all)
                for (xx, Ws, Wm) in ((rgb_all, W_s1, W_m1),
                                     (x_all, W_s2, W_m2)):
                    for c in range(NCH):
                        xm = p2f.tile([128, tl], BF, tag="xm",
                                      name=f"xm2_{col0}_{Ws.name}_{c}")
                        nc.vector.tensor_mul(xm[:], xx[:, c, cs],
                                             msk_all[:, cs])
                        nc.vector.tensor_reduce(
                            Ws[:, c, 2 * ti:2 * ti + nwt, :],
                            xm[:].rearrange("p (w n) -> p w n", n=N),
                            mybir.AxisListType.X, OP.add)
                        nc.vector.tensor_add(xm[:], xm[:], neg_all[:, cs])
                        nc.vector.tensor_reduce(
                            Wm[:, c, 2 * ti:2 * ti + nwt, :],
                            xm[:].rearrange("p (w n) -> p w n", n=N),
                            mybir.AxisListType.X, OP.max)
                col0 += tl

        # ===== PHASE 3: collectives (kicked first) + spatial path =====
        with tc.tile_pool(name="p3a", bufs=1) as p3a:
          with tc.tile_pool(name="w3", bufs=1) as wp3, \
               tc.tile_pool(name="p3", bufs=2) as p3, \
               tc.tile_pool(name="p3f", bufs=2) as p3f, \
               tc.tile_pool(name="zps", bufs=6, space="PSUM") as zps, \
               tc.tile_pool(name="sps", bufs=2, space="PSUM") as sps:
              cw1wTs = load_w(wp3, [128, 24, 4 * C // NCORE],
                              cw1wTs_d[:].rearrange("(c p) m -> p c m", p=128),
                              "cw1")
              cw2wTs = load_w(wp3, [128, 24, 2 * C // NCORE],
                              cw2wTs_d[:].rearrange("(c p) m -> p c m", p=128),
                              "cw2")
              sw1wT = load_w(wp3, [128, 2 * NCH, C], sw1wT_d[:].rearrange(
                  "(c p) m -> p c m", p=128), "sw1")
              sw2wT = cp.tile([128, NCH, 2], BF)
              nc.sync.dma_start(sw2wT[:], r6(sw2wT_d[:]))
              ident128 = p3a.tile([128, 128], F32)
              nc.vector.tensor_copy(ident128[:], identf[:])
              s01_all = p3a.tile([2, NTOK], F32)
              s1_row = p3a.tile([1, NTOK], F32)

              # ---- combine windows -> images, kick AllReduces ASAP ----
              imb = p3a.tile([128, 4, NW], F32)
              inb = p3a.tile([128, 4, NW], F32)
              for i in range(4):
                  r = stg.tile([1, NW], F32, tag="imrow")
                  nc.sync.dma_start(r[:], imgmask_d[i:i + 1, :])
                  nc.gpsimd.partition_broadcast(imb[:, i, :], r[:])
                  r2 = stg.tile([1, NW], F32, tag="imrow")
                  nc.sync.dma_start(r2[:], imgneg_d[i:i + 1, :])
                  nc.gpsimd.partition_broadcast(inb[:, i, :], r2[:])
              stat_s = p3a.tile([128, NCH, 2, 4], F32)
              stat_m = p3a.tile([128, NCH, 2, 4], F32)
              for k, Wt in ((0, W_s1), (1, W_s2)):
                  for i in range(4):
                      t = p3.tile([128, NCH, NW], F32, tag="cmb",
                                  name=f"cmb_{k}_{i}")
                      nc.vector.tensor_mul(t[:], Wt[:, :, 0:NW, 0],
                                           imb[:, i, None, :].broadcast_to(
                                               [128, NCH, NW]))
                      nc.vector.tensor_reduce(stat_s[:, :, k, i:i + 1], t[:],
                                              mybir.AxisListType.X, OP.add)
              for k, Wt in ((0, W_m1), (1, W_m2)):
                  for i in range(4):
                      t = p3.tile([128, NCH, NW], F32, tag="cmb",
                                  name=f"cmbm_{k}_{i}")
                      nc.vector.tensor_mul(t[:], Wt[:, :, 0:NW, 0],
                                           imb[:, i, None, :].broadcast_to(
                                               [128, NCH, NW]))
                      nc.vector.tensor_add(t[:], t[:],
                                           inb[:, i, None, :].broadcast_to(
                                               [128, NCH, NW]))
                      nc.vector.tensor_reduce(stat_m[:, :, k, i:i + 1], t[:],
                                              mybir.AxisListType.X, OP.max)
              nc.sync.dma_start(csum_in[:],
                                stat_s[:].rearrange("p a b c -> p (a b c)"))
              nc.sync.dma_start(cmax_in[:],
                                stat_m[:].rearrange("p a b c -> p (a b c)"))
              nc.gpsimd.collective_compute("AllReduce", OP.add,
                                           replica_groups=[core_ids],
                                           ins=[csum_in[:]], outs=[csum_out[:]])
              nc.gpsimd.collective_compute("AllReduce", OP.max,
                                           replica_groups=[core_ids],
                                           ins=[cmax_in[:]], outs=[cmax_out[:]])

              # ---- spatial sw path (PE work overlapping collectives) ----
              col0 = 0
              for ti, tl in enumerate(P34_TILES):
                  cs = slice(col0, col0 + tl)
                  zpsl = [zps.tile([128, tl], F32, tag="zp",
                                   name=f"zp_{col0}_{m}") for m in range(NCH)]
                  for mc in range(NCH):
                      for kc in range(2 * NCH):
                          rhs = (rgb_all[:, kc, cs] if kc < NCH
                                 else x_all[:, kc - NCH, cs])
                          nc.tensor.matmul(zpsl[mc][:],
                                           sw1wT[:, kc, mc * 128:(mc + 1) * 128],
                                           rhs, start=(kc == 0),
                                           stop=(kc == 2 * NCH - 1))
                  z_r = p3f.tile([128, NCH, tl], BF, tag="z_r", name=f"zr_{col0}")
                  for mc in range(NCH):
                      nc.scalar.activation(z_r[:, mc, :], zpsl[mc][:], AF.Relu,
                                           bias=sw1b_t[:, mc:mc + 1], scale=1.0)
                  sps_t = sps.tile([2, tl], F32, tag="sp", name=f"sp_{col0}")
                  for kc in range(NCH):
                      nc.tensor.matmul(sps_t[:], sw2wT[:, kc, :], z_r[:, kc, :],
                                       start=(kc == 0), stop=(kc == NCH - 1))
                  nc.vector.tensor_scalar_add(s01_all[:, cs], sps_t[:],
                                              sw2b_t[:])
                  col0 += tl
              # sigmoid(s)/2, split row 1 to partition 0
              nc.scalar.activation(s01_all[:], s01_all[:], AF.Sigmoid)
              nc.vector.tensor_scalar_mul(s01_all[:], s01_all[:], 0.5)
              nc.sync.dma_start(s1_row[:], s01_all[1:2, :])

              # channel MLP (sharded): ycat rhs [128, 24, 4]
              ycat_f = p3a.tile([128, 24, 4], F32)
              cso4 = csum_out[:].rearrange("p (c k i) -> p c k i", k=2, i=4)
              cmo4 = cmax_out[:].rearrange("p (c k i) -> p c k i", k=2, i=4)
              nc.sync.dma_start(ycat_f[:, 0:6, :], cso4[:, :, 0, :])
              nc.sync.dma_start(ycat_f[:, 6:12, :], cso4[:, :, 1, :])
              nc.sync.dma_start(ycat_f[:, 12:18, :], cmo4[:, :, 0, :])
              nc.sync.dma_start(ycat_f[:, 18:24, :], cmo4[:, :, 1, :])
              ycat_r = p3a.tile([128, 24, 4], BF)
              nc.vector.tensor_copy(ycat_r[:], ycat_f[:])
              z1sb = p3a.tile([128, 3, 4], BF)
              for mc in range(3):
                  ps = sps.tile([128, 4], F32, tag="sp", name=f"z1ps_{mc}")
                  for kc in range(24):
                      nc.tensor.matmul(ps[:], cw1wTs[:, kc, mc * 128:(mc + 1) * 128],
                                       ycat_r[:, kc, :], start=(kc == 0),
                                       stop=(kc == 23))
                  nc.scalar.activation(z1sb[:, mc, :], ps[:], AF.Relu,
                                       bias=cw1bs_t[:, mc:mc + 1], scale=1.0)
              z1f32 = p3a.tile([128, 3, 4], F32)
              nc.vector.tensor_copy(z1f32[:], z1sb[:])
              nc.sync.dma_start(z1_in[:].rearrange("(m p) f -> p m f", p=128),
                                z1f32[:])
              nc.gpsimd.collective_compute("AllGather", OP.bypass,
                                           replica_groups=[core_ids],
                                           ins=[z1_in[:]], outs=[z1_out[:]])
              z1f = p3a.tile([128, 24, 4], F32)
              nc.sync.dma_start(z1f[:],
                                z1_out[:].rearrange("(c p) f -> p c f", p=128))
              z1r = p3a.tile([128, 24, 4], BF)
              nc.vector.tensor_copy(z1r[:], z1f[:])
              z2sb = p3a.tile([128, 2, 4], F32)
              nc.any.memset(z2sb[:], 0.0)
              for mc, msz in ((0, 128), (1, 64)):
                  ps = sps.tile([128, 4], F32, tag="sp", name=f"z2ps_{mc}")
                  for kc in range(24):
                      nc.tensor.matmul(ps[0:msz, :],
                                       cw2wTs[:, kc, mc * 128:mc * 128 + msz],
                                       z1r[:, kc, :], start=(kc == 0),
                                       stop=(kc == 23))
                  nc.vector.tensor_scalar_add(z2sb[0:msz, mc, :], ps[0:msz, :],
                                              cw2bs_t[0:msz, mc:mc + 1])
              nc.sync.dma_start(z2_in[0:128, :], z2sb[:, 0, :])
              nc.sync.dma_start(z2_in[128:192, :], z2sb[0:64, 1, :])
              nc.gpsimd.collective_compute("AllGather", OP.bypass,
                                           replica_groups=[core_ids],
                                           ins=[z2_in[:]], outs=[z2_out[:]])
              y_f = p3a.tile([128, 12, 4], F32)
              nc.sync.dma_start(y_f[:],
                                z2_out[:].rearrange("(c p) f -> p c f", p=128))
              nc.scalar.activation(y_f[:], y_f[:], AF.Sigmoid)
              nc.vector.tensor_scalar_mul(y_f[:], y_f[:], 0.5)
              # transpose per chunk to [4, 128] bf16 for P4 matmuls
              cw0T = p3a.tile([4, NCH, 128], BF)   # y[:, :C]  (scales x1 -> out2)
              cw1T = p3a.tile([4, NCH, 128], BF)   # y[:, C:]  (scales x2 -> out1)
              for c in range(NCH):
                  for (dstt, src) in ((cw0T, y_f[:, c, :]),
                                      (cw1T, y_f[:, 6 + c, :])):
                      pstt = sps.tile([4, 128], F32, tag="sp",
                                      name=f"ct_{c}_{dstt.name}")
                      nc.tensor.transpose(pstt[:], src, ident128[:])
                      nc.scalar.copy(dstt[:, c, :], pstt[:])
              # rows of sw path as bf16 for rank-1 fold into P4 psums
              s0_bf = p3a.tile([1, NTOK], BF)
              nc.vector.tensor_copy(s0_bf[:], s01_all[0:1, :])
              s1_bf = p3a.tile([1, NTOK], BF)
              nc.vector.tensor_copy(s1_bf[:], s1_row[:])

              # ============ PHASE 4 (merged into phase-3 scope) ============
              p4 = p3
              cwp = zps
              col0 = 0
              for tl in P34_TILES:
                  cs = slice(col0, col0 + tl)
                  imsl = p4.tile([4, tl], BF, tag="imsr", name=f"ims_{col0}")
                  nc.sync.dma_start(imsl[:], imgsel_d[:, cs])
                  o1 = p4.tile([128, NCH, tl], BF, tag="o1", name=f"o1_{col0}")
                  o2 = p4.tile([128, NCH, tl], BF, tag="o2", name=f"o2_{col0}")
                  for c in range(NCH):
                      pc0 = cwp.tile([128, tl], F32, tag="zp",
                                     name=f"c0_{col0}_{c}")
                      nc.tensor.matmul(pc0[:], cw0T[:, c, :], imsl[:],
                                       start=True, stop=False)
                      nc.tensor.matmul(pc0[:], ones_row[:], s0_bf[:, cs],
                                       start=False, stop=True)
                      pc1 = cwp.tile([128, tl], F32, tag="zp",
                                     name=f"c1_{col0}_{c}")
                      nc.tensor.matmul(pc1[:], cw1T[:, c, :], imsl[:],
                                       start=True, stop=False)
                      nc.tensor.matmul(pc1[:], ones_row[:], s1_bf[:, cs],
                                       start=False, stop=True)
                      t0 = p4.tile([128, tl], BF, tag="t0", name=f"t0_{col0}_{c}")
                      nc.vector.tensor_mul(t0[:], pc1[:], x_all[:, c, cs])
                      nc.vector.tensor_add(o1[:, c, :], rgb_all[:, c, cs], t0[:])
                      t1 = p4.tile([128, tl], BF, tag="t1", name=f"t1_{col0}_{c}")
                      nc.vector.tensor_mul(t1[:], pc0[:], rgb_all[:, c, cs])
                      nc.vector.tensor_add(o2[:, c, :], x_all[:, c, cs], t1[:])
                  nc.sync.dma_start(r6(out1_d[:])[:, :, cs], o1[:])
                  nc.sync.dma_start(r6(out2_d[:])[:, :, cs], o2[:])
                  col0 += tl

    nc.compile()
    return nc


def _windowize(x):
    # [B, 64, 64, C] -> [104, 196, C] padded windows
    Bp = np.zeros((B, 70, 70, C), x.dtype)
    Bp[:, :64, :64, :] = x
    w = Bp.reshape(B, GRID, WIN, GRID, WIN, C).transpose(0, 1, 3, 2, 4, 5)
    w = w.reshape(NWIN_TOT, N, C)
    out = np.zeros((NCORE * NW, N, C), x.dtype)
    out[:NWIN_TOT] = w
    return out


def _unwindowize(perwin):
    # [104, 196, C] -> [B, 64, 64, C]
    w = perwin[:NWIN_TOT].reshape(B, GRID, GRID, WIN, WIN, C)
    w = w.transpose(0, 1, 3, 2, 4, 5).reshape(B, 70, 70, C)
    return np.ascontiguousarray(w[:, :64, :64, :])


def kernel(rgb_embedding, x_embedding, norm1_w, norm1_b, qkv_w, qkv_b,
           rel_pos_h, rel_pos_w, proj_w, proj_b, norm2_w, norm2_b,
           fc1_w, fc1_b, fc2_w, fc2_b, cw1_w, cw1_b, cw2_w, cw2_b,
           sw1_w, sw1_b, sw2_w, sw2_b):
    if "nc" not in _CACHE:
        _CACHE["nc"] = _build()
    nc = _CACHE["nc"]

    f32 = lambda a: np.ascontiguousarray(a, dtype=np.float32)
    bf = lambda a: np.ascontiguousarray(np.asarray(a, dtype=np.float32)
                                        .astype(NPBF))
    f8w = lambda a: np.ascontiguousarray(
        np.clip(np.asarray(a, np.float32), -240, 240)
        .astype(ml_dtypes.float8_e4m3))
    xw = _windowize(f32(x_embedding))        # [104, 196, C]
    rw = _windowize(f32(rgb_embedding))
    vm = np.zeros((NCORE * NW, N), np.float32)
    vh = np.minimum(np.maximum(64 - np.arange(GRID) * WIN, 0), WIN)
    wm = np.zeros((GRID, GRID, WIN, WIN), np.float32)
    for a in range(GRID):
        for b in range(GRID):
            wm[a, b, :vh[a], :vh[b]] = 1.0
    vm[:NWIN_TOT] = np.tile(wm.reshape(GRID * GRID, N), (B, 1))
    win_img = np.full(NCORE * NW, -1, np.int64)
    win_img[:NWIN_TOT] = np.arange(NWIN_TOT) // (GRID * GRID)

    qkb = f32(qkv_b[:2 * C]).copy()
    qkb[:C] *= SCALE
    tab = np.concatenate([f32(rel_pos_h)[::-1], f32(rel_pos_w)[::-1]], axis=0)
    tab = np.ascontiguousarray(tab.T) * (1.0 / SCALE)
    cw1s = f32(cw1_w).T.copy()              # [4C(k), 4C(m)]
    cw1s[:2 * C, :] *= 1.0 / (HH * WW)      # fold avg divisor
    shared = dict(
        qkwT=bf(f32(qkv_w)[:2 * C].T), qkb=qkb,
        vwT=bf(f32(qkv_w)[2 * C:].T), vb=f32(qkv_b[2 * C:]).copy(),
        projwT=bf(f32(proj_w).T), projb=f32(proj_b),
        n1w=f32(norm1_w), n1b=f32(norm1_b), n2w=f32(norm2_w), n2b=f32(norm2_b),
        tab=bf(tab),
        fc1w8=f8w(f32(fc1_w).T.reshape(3, 2, 128, DFF)
                  .transpose(2, 0, 1, 3)), fc1b=f32(fc1_b),
        fc2wT=bf(f32(fc2_w).T), fc2b=f32(fc2_b),
        sw1wT=bf(f32(sw1_w).T), sw1b=f32(sw1_b),
        sw2wT=bf(f32(sw2_w).T), sw2b=f32(sw2_b),
    )
    cw2s = np.ascontiguousarray(f32(cw2_w).T)  # [4C, 2C]
    in_maps = []
    for c in range(NCORE):
        sl = slice(c * NW, (c + 1) * NW)
        xT = bf(xw[sl].reshape(NTOK, C).T)
        rT = bf(rw[sl].reshape(NTOK, C).T)
        mrow = vm[sl].reshape(1, NTOK).copy()
        imgm = np.zeros((4, NW), np.float32)
        imsel = np.zeros((4, NTOK), np.float32)
        for wloc in range(NW):
            im = win_img[c * NW + wloc]
            if im >= 0:
                imgm[im, wloc] = 1.0
                imsel[im, wloc * N:(wloc + 1) * N] = 1.0
        m = dict(shared)
        m.update(
            xT=xT, rgbT=rT, mask=bf(mrow),
            imgmask=imgm, imgneg=(imgm - 1.0) * 1e30, imgsel=bf(imsel),
            cw1wTs=bf(cw1s[:, c * 384:(c + 1) * 384]),
            cw1bs=f32(cw1_b[c * 384:(c + 1) * 384]).copy(),
            cw2wTs=bf(cw2s[:, c * 192:(c + 1) * 192]),
            cw2bs=f32(cw2_b[c * 192:(c + 1) * 192]).copy(),
        )
        in_maps.append(m)

    trace = bool(os.environ.get("KERNEL_TRACE"))
    res = run_bass_kernel_spmd(nc, in_maps, list(range(NCORE)), trace=trace)
    if trace:
        _CACHE["exec_time_ns"] = res.exec_time_ns
    o1 = np.zeros((NCORE * NW, N, C), np.float32)
    o2 = np.zeros((NCORE * NW, N, C), np.float32)
    for c in range(NCORE):
        sl = slice(c * NW, (c + 1) * NW)
        o1[sl] = res.results[c]["out1T"].astype(np.float32).T.reshape(NW, N, C)
        o2[sl] = res.results[c]["out2T"].astype(np.float32).T.reshape(NW, N, C)
    rgb_out = _unwindowize(o1)
    x_out = _unwindowize(o2)
    return rgb_out, x_out


# revision 43
# speedup vs baseline: 1.0095x; 1.0069x over previous
"""SAM-block (windowed attention + MLP) + FRM fusion on 8 TRN2 NeuronCores.

v2: bf16 datapath (fp32 PSUM accumulation), SBUF-resident intermediates,
batched rel-pos gather DMAs, per-window score matmuls, bf16 outputs.

Self-contained: shards the 100 attention windows over 8 cores (13/core),
runs one SPMD Bass program via run_bass_kernel_spmd, reassembles on host.
Device layout: [C(partitions), tokens(free)] for projections; attention
blocks operate on window pairs (392 tokens) with tokens on partitions.
"""
import os
import numpy as np
import ml_dtypes
import concourse.bass as bass
import concourse.bacc as bacc
import concourse.mybir as mybir
from concourse import tile
from concourse.masks import make_identity
from concourse.bass_utils import run_bass_kernel_spmd

F32 = mybir.dt.float32
BF = mybir.dt.bfloat16
F8 = mybir.dt.float8e4
AF = mybir.ActivationFunctionType
OP = mybir.AluOpType
NPBF = ml_dtypes.bfloat16

# problem constants
B, HH, WW, C = 4, 64, 64, 768
WIN, NH, HD = 14, 12, 64
S = WIN
N = S * S                  # 196 tokens / window
GRID = 5                   # 5x5 windows per image (64 -> 70 padded)
NWIN_TOT = B * GRID * GRID  # 100
NCORE = 8
NW = 13                    # windows per core (104 slots, 4 dummy)
NTOK = NW * N              # 2548
NT = 2 * N                 # 392 = pair block
NCH = C // 128             # 6
DFF = 4 * C
SCALE = HD ** -0.5
EPS = 1e-6
# 6 pair blocks + 1 single-window block (window 12)
BLOCKS = [(i * NT, 2) for i in range(6)] + [(6 * NT, 1)]
P2_TILES = [392] * 6 + [196]
P34_TILES = [392] * 6 + [196]

_CACHE = {}


def _build():
    nc = bacc.Bacc("TRN2", target_bir_lowering=False, debug=False)
    dt_in = {}

    def din(name, shape, dt=BF):
        dt_in[name] = nc.dram_tensor(name, shape, dt, kind="ExternalInput")
        return dt_in[name]

    xT_d = din("xT", [C, NTOK])
    rgbT_d = din("rgbT", [C, NTOK])
    mask_d = din("mask", [1, NTOK], BF)
    imgmask_d = din("imgmask", [4, NW], F32)
    imgneg_d = din("imgneg", [4, NW], F32)
    imgsel_d = din("imgsel", [4, NTOK], BF)
    qkwT_d = din("qkwT", [C, 2 * C])
    qkb_d = din("qkb", [2 * C], F32)
    vwT_d = din("vwT", [C, C])
    vb_d = din("vb", [C], F32)
    projwT_d = din("projwT", [C, C])
    projb_d = din("projb", [C], F32)
    n1w_d = din("n1w", [C], F32)
    n1b_d = din("n1b", [C], F32)
    n2w_d = din("n2w", [C], F32)
    n2b_d = din("n2b", [C], F32)
    tab_d = din("tab", [HD, 54])
    fc1w8_d = din("fc1w8", [128, 3, 2, DFF], F8)
    fc1b_d = din("fc1b", [DFF], F32)
    fc2wT_d = din("fc2wT", [DFF, C])
    fc2b_d = din("fc2b", [C], F32)
    sw1wT_d = din("sw1wT", [2 * C, C])
    sw1b_d = din("sw1b", [C], F32)
    sw2wT_d = din("sw2wT", [C, 2])
    sw2b_d = din("sw2b", [2], F32)
    cw1wTs_d = din("cw1wTs", [4 * C, 4 * C // NCORE])
    cw1bs_d = din("cw1bs", [4 * C // NCORE], F32)
    cw2wTs_d = din("cw2wTs", [4 * C, 2 * C // NCORE])
    cw2bs_d = din("cw2bs", [2 * C // NCORE], F32)
    out1_d = nc.dram_tensor("out1T", [C, NTOK], BF, kind="ExternalOutput")
    out2_d = nc.dram_tensor("out2T", [C, NTOK], BF, kind="ExternalOutput")

    core_ids = list(range(NCORE))
    r6 = lambda ap: ap.rearrange("(c p) n -> p c n", p=128)
    rcol = lambda ap: ap.rearrange("(c p) -> p c", p=128)

    with tile.TileContext(nc) as tc:
      with tc.tile_pool(name="dram", bufs=1, space="DRAM") as dramp, \
           tc.tile_pool(name="fbp", bufs=2, space="DRAM") as fbp, \
           tc.tile_pool(name="cst", bufs=1) as cp, \
           tc.tile_pool(name="res", bufs=1) as resp, \
           tc.tile_pool(name="stg", bufs=2) as stg:
        # ---------- DRAM scratch (collectives only) ----------
        csum_in = dramp.tile([128, 48], F32)
        csum_out = dramp.tile([128, 48], F32, addr_space="Shared")
        cmax_in = dramp.tile([128, 48], F32)
        cmax_out = dramp.tile([128, 48], F32, addr_space="Shared")
        z1_in = dramp.tile([4 * C // NCORE, 4], F32)
        z1_out = dramp.tile([4 * C, 4], F32, addr_space="Shared")
        z2_in = dramp.tile([2 * C // NCORE, 4], F32)
        z2_out = dramp.tile([2 * C, 4], F32, addr_space="Shared")

        # ---------- SBUF-resident activations ----------
        # x_all holds x -> (in-place) x2a = x+attn -> (in-place) x2 = x2a+mlp
        x_all = resp.tile([128, NCH, NTOK], BF)
        nc.sync.dma_start(x_all[:], r6(xT_d[:]))

        # ---------- persistent constants ----------
        identf = stg.tile([128, 128], F32, tag="st1")
        make_identity(nc, identf)
        ident98 = cp.tile([98, 98], BF)
        nc.vector.tensor_copy(ident98[:], identf[0:98, 0:98])

        def load_rows(src, n=C):
            t = cp.tile([128, n // 128], F32, name="rows_" + src.tensor.name)
            nc.sync.dma_start(t[:], rcol(src))
            return t

        qkb_t = load_rows(qkb_d[:], 2 * C)
        projb_t = load_rows(projb_d[:])
        n1w_t = load_rows(n1w_d[:])
        n1b_t = load_rows(n1b_d[:])
        n2w_t = load_rows(n2w_d[:])
        n2b_t = load_rows(n2b_d[:])
        fc1b_t = load_rows(fc1b_d[:], DFF)
        fc2b_t = load_rows(fc2b_d[:])
        sw1b_t = load_rows(sw1b_d[:])
        sw2b_t = cp.tile([2, 1], F32)
        nc.sync.dma_start(sw2b_t[:, 0], sw2b_d[:])
        cw1bs_t = load_rows(cw1bs_d[:], 4 * C // NCORE)
        cw2bs_t = cp.tile([128, 2], F32)
        nc.any.memset(cw2bs_t[:], 0.0)
        nc.sync.dma_start(cw2bs_t[0:128, 0], cw2bs_d[0:128])
        nc.sync.dma_start(cw2bs_t[0:64, 1], cw2bs_d[128:192])
        vb_t = load_rows(vb_d[:])
        tab2 = cp.tile([128, 2, 54], BF)
        nc.any.memset(tab2[:], 0.0)
        nc.sync.dma_start(tab2[0:64, 0, :], tab_d[:])
        nc.sync.dma_start(tab2[64:128, 1, :], tab_d[:])
        ones_f = stg.tile([128, 1], F32, tag="st1")
        nc.any.memset(ones_f[:], 1.0)
        ones_r = cp.tile([128, 1], BF)
        nc.vector.tensor_copy(ones_r[:], ones_f[:])
        ones_row = cp.tile([1, 128], BF)
        nc.any.memset(ones_row[:], 1.0)
        mask_bf = cp.tile([1, NTOK], BF)
        nc.sync.dma_start(mask_bf[:], mask_d[:])

        def load_w(pool_, shape3, src_ap, nm, dt=BF):
            # weights load directly (no staging/round)
            r = pool_.tile(shape3, dt, name="w_" + nm)
            nc.sync.dma_start(r[:], src_ap)
            return r

        # ==================== PHASE 1: attention ====================
        with tc.tile_pool(name="w1", bufs=1) as wp1, \
             tc.tile_pool(name="p1", bufs=1) as p1, \
             tc.tile_pool(name="p1b", bufs=2) as p1b, \
             tc.tile_pool(name="p1r", bufs=1) as p1r, \
             tc.tile_pool(name="p1s", bufs=2) as p1s, \
             tc.tile_pool(name="p1f", bufs=2) as p1f, \
             tc.tile_pool(name="ln", bufs=2, space="PSUM") as psln, \
             tc.tile_pool(name="gen", bufs=3, space="PSUM") as psg, \
             tc.tile_pool(name="pst", bufs=1, space="PSUM") as pst, \
             tc.tile_pool(name="psa", bufs=2, space="PSUM") as psa:
            qkwT = load_w(wp1, [128, NCH, 2 * C], r6(qkwT_d[:]), "qk")
            vwT = load_w(wp1, [128, NCH, C], r6(vwT_d[:]), "v")
            projwT = load_w(wp1, [128, NCH, C], r6(projwT_d[:]), "pj")

            kzAB = [p1.tile([128, NH, NT], BF, tag=f"kz{i}", name=f"kz{i}")
                    for i in range(2)]
            nc.any.memset(kzAB[0][:], 0.0)
            nc.any.memset(kzAB[1][:], 0.0)

            for blk, (c0, nw) in enumerate(BLOCKS):
                NTb = nw * N
                cols = slice(c0, c0 + NTb)
                kzt = kzAB[blk % 2]

                # ---- LN1 stats ----
                s1 = psln.tile([1, NT], F32, tag="lnp")
                s2 = psln.tile([1, NT], F32, tag="lnp")
                for c in range(NCH):
                    nc.tensor.matmul(s1[:, 0:NTb], ones_r[:], x_all[:, c, cols],
                                     start=(c == 0), stop=(c == NCH - 1))
                for c in range(NCH):
                    xsqc = p1f.tile([128, NT], BF, tag="xsqc")
                    nc.vector.tensor_mul(xsqc[:, 0:NTb], x_all[:, c, cols],
                                         x_all[:, c, cols])
                    nc.tensor.matmul(s2[:, 0:NTb], ones_r[:], xsqc[:, 0:NTb],
                                     start=(c == 0), stop=(c == NCH - 1))
                mrow_t = p1r.tile([1, NT], F32, tag="r1")
                nc.scalar.activation(mrow_t[:, 0:NTb], s1[:, 0:NTb], AF.Copy,
                                     scale=1.0 / C)
                var = p1r.tile([1, NT], F32, tag="r2")
                nc.vector.tensor_mul(var[:, 0:NTb], mrow_t[:, 0:NTb],
                                     mrow_t[:, 0:NTb])
                e2 = p1r.tile([1, NT], F32, tag="r3")
                nc.scalar.activation(e2[:, 0:NTb], s2[:, 0:NTb], AF.Copy,
                                     scale=1.0 / C)
                nc.vector.tensor_sub(var[:, 0:NTb], e2[:, 0:NTb], var[:, 0:NTb])
                nc.vector.tensor_scalar_add(var[:, 0:NTb], var[:, 0:NTb], EPS)
                sd = p1r.tile([1, NT], F32, tag="r4")
                nc.scalar.activation(sd[:, 0:NTb], var[:, 0:NTb], AF.Sqrt)
                inv = p1r.tile([1, NT], F32, tag="r5")
                nc.vector.reciprocal(inv[:, 0:NTb], sd[:, 0:NTb])
                mrow_b = p1r.tile([1, NT], BF, tag="r6")
                nc.vector.tensor_copy(mrow_b[:, 0:NTb], mrow_t[:, 0:NTb])
                # fold the window-validity mask into inv (norm1_b == 0)
                inv_bf = p1r.tile([1, NT], BF, tag="r7")
                nc.vector.tensor_mul(inv_bf[:, 0:NTb], inv[:, 0:NTb],
                                     mask_bf[:, cols])
                m_b = p1f.tile([128, NT], BF, tag="m_b")
                nc.gpsimd.partition_broadcast(m_b[:, 0:NTb], mrow_b[:, 0:NTb])
                inv_b = p1f.tile([128, NT], BF, tag="inv_b")
                nc.gpsimd.partition_broadcast(inv_b[:, 0:NTb], inv_bf[:, 0:NTb])
                lnx = p1b.tile([128, NCH, NT], BF, tag="lnx")
                for c in range(NCH):
                    u = p1s.tile([128, NT], BF, tag="u")
                    nc.vector.tensor_sub(u[:, 0:NTb], x_all[:, c, cols],
                                         m_b[:, 0:NTb])
                    nc.vector.tensor_mul(u[:, 0:NTb], u[:, 0:NTb],
                                         inv_b[:, 0:NTb])
                    nc.scalar.activation(lnx[:, c, 0:NTb], u[:, 0:NTb],
                                         AF.Identity, bias=n1b_t[:, c:c + 1],
                                         scale=n1w_t[:, c:c + 1])

                # ---- qk projection ----
                qT = p1b.tile([128, NCH, NT], BF, tag="qT")
                for mc in range(12):
                    ps = psg.tile([128, NT], F32, tag="gen")
                    for kc in range(NCH):
                        nc.tensor.matmul(ps[:, 0:NTb],
                                         qkwT[:, kc, mc * 128:(mc + 1) * 128],
                                         lnx[:, kc, 0:NTb], start=(kc == 0),
                                         stop=(kc == NCH - 1))
                    if mc < NCH:
                        nc.scalar.activation(qT[:, mc, 0:NTb], ps[:, 0:NTb],
                                             AF.Identity,
                                             bias=qkb_t[:, mc:mc + 1],
                                             scale=SCALE)
                    else:
                        j = mc - NCH
                        nc.scalar.activation(kzt[0:64, 2 * j, 0:NTb],
                                             ps[0:64, 0:NTb], AF.Identity,
                                             bias=qkb_t[0:64, mc:mc + 1],
                                             scale=1.0)
                        nc.scalar.activation(kzt[64:128, 2 * j + 1, 0:NTb],
                                             ps[64:128, 0:NTb], AF.Identity,
                                             bias=qkb_t[64:128, mc:mc + 1],
                                             scale=1.0)

                # ---- f features (rel-pos), batched ----
                # layout [p, qc, w, h, t] so (w,h) is one contiguous dim
                fall = p1b.tile([98, 2, 2, NH, 54], BF, tag="fall",
                                name=f"fall_{blk}")
                for qc in range(2):
                    for w in range(nw):
                        ps1 = psg.tile([98, 9 * 54], F32, tag="gen")
                        ps2 = psg.tile([98, 3 * 54], F32, tag="gen")
                        for h in range(NH):
                            dst = ps1[:, h * 54:h * 54 + 54] if h < 9 else \
                                ps2[:, (h - 9) * 54:(h - 9) * 54 + 54]
                            lhsT = qT[:, h // 2, w * N + qc * 98: w * N + qc * 98 + 98]
                            nc.tensor.matmul(dst, lhsT, tab2[:, h % 2, :],
                                             start=True, stop=True)
                        nc.scalar.copy(fall[:, qc, w, 0:9, :],
                                       ps1[:].rearrange("p (h t) -> p h t", t=54))
                        nc.scalar.copy(fall[:, qc, w, 9:12, :],
                                       ps2[:].rearrange("p (h t) -> p h t", t=54))
                fd = fbp.tile([98, 2, 2, NH, 54], BF, tag="fb", name=f"fd_{blk}")
                nc.sync.dma_start(fd[:], fall[:])
                gh = p1f.tile([98, 2, 2, NH, 14], BF, tag="gh", name=f"gh_{blk}")
                gw = p1f.tile([98, 2, 2, NH, 14], BF, tag="gw", name=f"gw_{blk}")
                for qc in range(2):
                    for g in range(7):
                        ih = qc * 7 + g
                        nc.sync.dma_start(
                            gh[g * 14:(g + 1) * 14, qc, :, :, :],
                            fall[g * 14:(g + 1) * 14, qc, :, :,
                                 13 - ih:27 - ih])
                        src_ap = bass.AP(fd[:].tensor,
                                         (g * 14) * 2592 + qc * 1296 + 40,
                                         [[2591, 14], [54, 2 * NH], [1, 14]])
                        nc.sync.dma_start(
                            gw[g * 14:(g + 1) * 14, qc, :, :, :].rearrange(
                                "p w h k -> p (w h) k"), src_ap)

                # ---- v (token-major pairs) ----
                v_pair = [p1b.tile([98, NH, 2, 64], BF, tag=f"vp{i}",
                                   name=f"vp{i}_{blk}") for i in range(2)]
                for c4 in range(2 * nw):
                    ps_a = psg.tile([98, 512], F32, tag="gen")
                    ps_b = psg.tile([98, 256], F32, tag="gen")
                    for kc in range(NCH):
                        nc.tensor.matmul(ps_a[:], lnx[:, kc, c4 * 98:(c4 + 1) * 98],
                                         vwT[:, kc, 0:512], start=(kc == 0),
                                         stop=(kc == NCH - 1))
                    for kc in range(NCH):
                        nc.tensor.matmul(ps_b[:], lnx[:, kc, c4 * 98:(c4 + 1) * 98],
                                         vwT[:, kc, 512:768], start=(kc == 0),
                                         stop=(kc == NCH - 1))
                    nc.scalar.copy(
                        v_pair[c4 % 2][:, 0:8, c4 // 2, :],
                        ps_a[:].rearrange("p (h d) -> p h d", d=64))
                    nc.scalar.copy(
                        v_pair[c4 % 2][:, 8:12, c4 // 2, :],
                        ps_b[:].rearrange("p (h d) -> p h d", d=64))

                # ---- scores + softmax + transpose + AV ----
                attn_outT = p1b.tile([128, NCH, NT], BF, tag="attn_outT")
                GH = 2
                for grp in range(NH // GH):
                    P_rg = {}
                    for w in range(nw):
                        for qc in range(2):
                            sco = p1f.tile([98, GH, S, S], BF, tag="sco")
                            ps = psg.tile([98, GH, S, S], F32, tag="gen")
                            lhsT = qT[:, grp,
                                      w * N + qc * 98: w * N + qc * 98 + 98]
                            for hh in range(GH):
                                h = grp * GH + hh
                                nc.tensor.matmul(ps[:, hh, :, :], lhsT,
                                                 kzt[:, h, w * N:(w + 1) * N],
                                                 start=True, stop=True)
                            nc.vector.tensor_add(
                                sco[:], ps[:],
                                gh[:, qc, w, grp * GH:grp * GH + GH, :, None]
                                .broadcast_to([98, GH, S, S]))
                            nc.vector.tensor_add(
                                sco[:], sco[:],
                                gw[:, qc, w, grp * GH:grp * GH + GH, None, :]
                                .broadcast_to([98, GH, S, S]))
                            nc.scalar.activation(sco[:], sco[:], AF.Exp)
                            z = p1s.tile([98, GH, 1, 1], F32, tag="z")
                            nc.vector.tensor_reduce(z[:], sco[:],
                                                    mybir.AxisListType.XY, OP.add)
                            nc.vector.reciprocal(z[:], z[:])
                            prg = p1f.tile([98, GH, S, S], BF, tag=f"P{w}{qc}",
                                           name=f"P{w}{qc}_{blk}_{grp}")
                            for hh in range(GH):
                                nc.vector.tensor_scalar_mul(
                                    prg[:, hh, :, :], sco[:, hh, :, :],
                                    z[:, hh, 0, :])
                            P_rg[(w, qc)] = prg
                    for hh in range(GH):
                        h = grp * GH + hh
                        PT_sb = []
                        for kc in range(2):
                            psT = pst.tile([98, 392], BF, tag="psT")
                            for j, (w, qc) in enumerate(
                                    [(w, qc) for w in range(nw)
                                     for qc in range(2)]):
                                nc.tensor.transpose(
                                    psT[:, j * 98:(j + 1) * 98],
                                    P_rg[(w, qc)][:, hh, 7 * kc:7 * kc + 7, :],
                                    ident98[:])
                            sb = p1s.tile([98, 392], BF, tag="PTsb")
                            nc.scalar.copy(sb[:, 0:nw * N], psT[:, 0:nw * N])
                            PT_sb.append(sb)
                        psA = psa.tile([128, 392], F32, tag="psAV")
                        for kc in range(2):
                            nc.tensor.matmul(psA[0:64 * nw, 0:NTb],
                                             v_pair[kc][:, h, 0:nw, :],
                                             PT_sb[kc][:, 0:NTb], start=(kc == 0),
                                             stop=(kc == 1))
                        nc.scalar.activation(
                            attn_outT[(h % 2) * 64:(h % 2) * 64 + 64, h // 2,
                                      0:196],
                            psA[0:64, 0:196], AF.Identity,
                            bias=vb_t[(h % 2) * 64:(h % 2) * 64 + 64,
                                      h // 2:h // 2 + 1], scale=1.0)
                        if nw == 2:
                            nc.vector.tensor_scalar_add(
                                attn_outT[(h % 2) * 64:(h % 2) * 64 + 64, h // 2,
                                          196:392],
                                psA[64:128, 196:392],
                                vb_t[(h % 2) * 64:(h % 2) * 64 + 64,
                                     h // 2:h // 2 + 1])

                # ---- proj + residual -> x2a (SBUF) ----
                for mc in range(NCH):
                    ps = psg.tile([128, NT], F32, tag="gen")
                    for kc in range(NCH):
                        nc.tensor.matmul(ps[:, 0:NTb],
                                         projwT[:, kc, mc * 128:(mc + 1) * 128],
                                         attn_outT[:, kc, 0:NTb], start=(kc == 0),
                                         stop=(kc == NCH - 1))
                    tmp = p1s.tile([128, NT], BF, tag="projtmp")
                    nc.scalar.activation(tmp[:, 0:NTb], ps[:, 0:NTb], AF.Identity,
                                         bias=projb_t[:, mc:mc + 1], scale=1.0)
                    nc.vector.tensor_add(x_all[:, mc, cols], tmp[:, 0:NTb],
                                         x_all[:, mc, cols])

        # rgb_all allocated only now (frees 30KB/partition for phase 1)
        resp2_cm = tc.tile_pool(name="res2", bufs=1)
        resp2 = resp2_cm.__enter__()
        rgb_all = resp2.tile([128, NCH, NTOK], BF, name="rgb_all")
        # ============ PHASE 2: LN2 stat pre-pass + fp8 MLP ============
        m2_row = resp.tile([1, NTOK], BF, name="m2_row")
        i2_row = resp.tile([1, NTOK], BF, name="i2_row")
        msk_all = resp.tile([128, NTOK], BF, name="msk_all")
        nc.gpsimd.partition_broadcast(msk_all[:], mask_bf[:])
        neg_all = resp.tile([128, NTOK], BF, name="neg_all")
        nc.vector.tensor_scalar_add(neg_all[:], msk_all[:], -1.0)
        nc.vector.tensor_scalar_mul(neg_all[:], neg_all[:], 1e30)
        W_s1 = resp.tile([128, NCH, 14, 1], F32, name="W_s1")
        W_s2 = resp.tile([128, NCH, 14, 1], F32, name="W_s2")
        W_m1 = resp.tile([128, NCH, 14, 1], F32, name="W_m1")
        W_m2 = resp.tile([128, NCH, 14, 1], F32, name="W_m2")
        with tc.tile_pool(name="pre2", bufs=2) as pre2, \
             tc.tile_pool(name="psp", bufs=2, space="PSUM") as psp:
            # prefetch rgb for stats fusion + phases 3/4
            nc.sync.dma_start(rgb_all[:], r6(rgbT_d[:]))
            col0 = 0
            for tl in P2_TILES:
                cs = slice(col0, col0 + tl)
                s1 = psp.tile([1, tl], F32, tag="s1", name=f"pps1_{col0}")
                s2 = psp.tile([1, tl], F32, tag="s2", name=f"pps2_{col0}")
                for c in range(NCH):
                    nc.tensor.matmul(s1[:], ones_r[:], x_all[:, c, cs],
                                     start=(c == 0), stop=(c == NCH - 1))
                for c in range(NCH):
                    xsqc = pre2.tile([128, tl], BF, tag="xsqc",
                                     name=f"xsq2_{col0}_{c}")
                    nc.vector.tensor_mul(xsqc[:], x_all[:, c, cs],
                                         x_all[:, c, cs])
                    nc.tensor.matmul(s2[:], ones_r[:], xsqc[:],
                                     start=(c == 0), stop=(c == NCH - 1))
                mrow = pre2.tile([1, tl], F32, tag="r1", name=f"m2m_{col0}")
                nc.scalar.activation(mrow[:], s1[:], AF.Copy, scale=1.0 / C)
                var = pre2.tile([1, tl], F32, tag="r2", name=f"m2v_{col0}")
                nc.vector.tensor_mul(var[:], mrow[:], mrow[:])
                e2 = pre2.tile([1, tl], F32, tag="r3", name=f"m2e_{col0}")
                nc.scalar.activation(e2[:], s2[:], AF.Copy, scale=1.0 / C)
                nc.vector.tensor_sub(var[:], e2[:], var[:])
                nc.vector.tensor_scalar_add(var[:], var[:], EPS)
                sd = pre2.tile([1, tl], F32, tag="r4", name=f"m2s_{col0}")
                nc.scalar.activation(sd[:], var[:], AF.Sqrt)
                inv = pre2.tile([1, tl], F32, tag="r5", name=f"m2i_{col0}")
                nc.vector.reciprocal(inv[:], sd[:])
                nc.vector.tensor_copy(m2_row[:, cs], mrow[:])
                nc.vector.tensor_copy(i2_row[:, cs], inv[:])
                col0 += tl
        with tc.tile_pool(name="w2", bufs=1) as wp2, \
             tc.tile_pool(name="p2", bufs=2) as p2, \
             tc.tile_pool(name="p2f", bufs=2) as p2f, \
             tc.tile_pool(name="ps1p", bufs=2, space="PSUM") as ps1p, \
             tc.tile_pool(name="ps2p", bufs=6, space="PSUM") as ps2p:
            fc1w8 = load_w(wp2, [128, 3, 2, DFF], fc1w8_d[:], "fc1", dt=F8)
            fc2wT = load_w(wp2, [128, DFF // 128, C], r6(fc2wT_d[:]), "fc2")
            DR = mybir.MatmulPerfMode.DoubleRow
            col0 = 0
            for ti, tl in enumerate(P2_TILES):
                nwt = tl // N
                cs = slice(col0, col0 + tl)
                m_b = p2f.tile([128, tl], BF, tag="m_b", name=f"mb_{col0}")
                nc.gpsimd.partition_broadcast(m_b[:], m2_row[:, cs])
                inv_b = p2f.tile([128, tl], BF, tag="inv_b", name=f"ib_{col0}")
                nc.gpsimd.partition_broadcast(inv_b[:], i2_row[:, cs])
                ln28 = p2.tile([128, 3, 2, tl], F8, tag="ln28", name=f"l2_{col0}")
                for c in range(NCH):
                    u = p2.tile([128, tl], BF, tag="u", name=f"u_{col0}_{c}")
                    nc.vector.tensor_sub(u[:], x_all[:, c, cs], m_b[:])
                    nc.vector.tensor_mul(u[:], u[:], inv_b[:])
                    nc.scalar.activation(ln28[:, c // 2, c % 2, :], u[:],
                                         AF.Identity, bias=n2b_t[:, c:c + 1],
                                         scale=n2w_t[:, c:c + 1])
                h_bf = p2.tile([128, 24, tl], BF, tag="h8", name=f"h8_{col0}")
                psum2 = [ps2p.tile([128, tl], F32, tag="ps2",
                                   name=f"ps2_{col0}_{m}") for m in range(NCH)]
                for kt2 in range(12):
                    for jj in range(2):
                        m24 = kt2 * 2 + jj
                        ps1 = ps1p.tile([128, tl], F32, tag="ps1",
                                        name=f"ps1_{col0}_{m24}")
                        for kt in range(3):
                            nc.tensor.matmul(
                                ps1[:], fc1w8[:, kt, :, m24 * 128:(m24 + 1) * 128],
                                ln28[:, kt, :, :], start=(kt == 0),
                                stop=(kt == 2), perf_mode=DR)
                        nc.scalar.activation(h_bf[:, m24, :], ps1[:], AF.Gelu,
                                             bias=fc1b_t[:, m24:m24 + 1],
                                             scale=1.0)
                        for mc in range(NCH):
                            nc.tensor.matmul(
                                psum2[mc][:],
                                fc2wT[:, m24, mc * 128:(mc + 1) * 128],
                                h_bf[:, m24, :], start=(m24 == 0),
                                stop=(m24 == 23))
                for mc in range(NCH):
                    tmp = p2.tile([128, tl], BF, tag="fct", name=f"fct_{col0}_{mc}")
                    nc.scalar.activation(tmp[:], psum2[mc][:], AF.Identity,
                                         bias=fc2b_t[:, mc:mc + 1], scale=1.0)
                    nc.vector.tensor_add(x_all[:, mc, cs], tmp[:],
                                         x_all[:, mc, cs])
                # FRM per-window stats fused here (x1 = rgb, x2 = x_all)
                for (xx, Ws, Wm) in ((rgb_all, W_s1, W_m1),
                                     (x_all, W_s2, W_m2)):
                    for c in range(NCH):
                        xm = p2f.tile([128, tl], BF, tag="xm",
                                      name=f"xm2_{col0}_{Ws.name}_{c}")
                        nc.vector.tensor_mul(xm[:], xx[:, c, cs],
                                             msk_all[:, cs])
                        nc.vector.tensor_reduce(
                            Ws[:, c, 2 * ti:2 * ti + nwt, :],
                            xm[:].rearrange("p (w n) -> p w n", n=N),
                            mybir.AxisListType.X, OP.add)
                        nc.vector.tensor_add(xm[:], xm[:], neg_all[:, cs])
                        nc.vector.tensor_reduce(
                            Wm[:, c, 2 * ti:2 * ti + nwt, :],
                            xm[:].rearrange("p (w n) -> p w n", n=N),
                            mybir.AxisListType.X, OP.max)
                col0 += tl

        # ===== PHASE 3: collectives (kicked first) + spatial path =====
        with tc.tile_pool(name="p3a", bufs=1) as p3a:
          with tc.tile_pool(name="w3", bufs=1) as wp3, \
               tc.tile_pool(name="p3", bufs=2) as p3, \
               tc.tile_pool(name="p3f", bufs=2) as p3f, \
               tc.tile_pool(name="zps", bufs=6, space="PSUM") as zps, \
               tc.tile_pool(name="sps", bufs=2, space="PSUM") as sps:
              cw1wTs = load_w(wp3, [128, 24, 4 * C // NCORE],
                              cw1wTs_d[:].rearrange("(c p) m -> p c m", p=128),
                              "cw1")
              cw2wTs = load_w(wp3, [128, 24, 2 * C // NCORE],
                              cw2wTs_d[:].rearrange("(c p) m -> p c m", p=128),
                              "cw2")
              sw1wT = load_w(wp3, [128, 2 * NCH, C], sw1wT_d[:].rearrange(
                  "(c p) m -> p c m", p=128), "sw1")
              sw2wT = cp.tile([128, NCH, 2], BF)
              nc.sync.dma_start(sw2wT[:], r6(sw2wT_d[:]))
              ident128 = p3a.tile([128, 128], F32)
              nc.vector.tensor_copy(ident128[:], identf[:])
              s01_all = p3a.tile([2, NTOK], F32)
              s1_row = p3a.tile([1, NTOK], F32)

              # ---- combine windows -> images, kick AllReduces ASAP ----
              imb = p3a.tile([128, 4, NW], F32)
              inb = p3a.tile([128, 4, NW], F32)
              for i in range(4):
                  r = stg.tile([1, NW], F32, tag="imrow")
                  nc.sync.dma_start(r[:], imgmask_d[i:i + 1, :])
                  nc.gpsimd.partition_broadcast(imb[:, i, :], r[:])
                  r2 = stg.tile([1, NW], F32, tag="imrow")
                  nc.sync.dma_start(r2[:], imgneg_d[i:i + 1, :])
                  nc.gpsimd.partition_broadcast(inb[:, i, :], r2[:])
              stat_s = p3a.tile([128, NCH, 2, 4], F32)
              stat_m = p3a.tile([128, NCH, 2, 4], F32)
              for k, Wt in ((0, W_s1), (1, W_s2)):
                  for i in range(4):
                      t = p3.tile([128, NCH, NW], F32, tag="cmb",
                                  name=f"cmb_{k}_{i}")
                      nc.vector.tensor_mul(t[:], Wt[:, :, 0:NW, 0],
                                           imb[:, i, None, :].broadcast_to(
                                               [128, NCH, NW]))
                      nc.vector.tensor_reduce(stat_s[:, :, k, i:i + 1], t[:],
                                              mybir.AxisListType.X, OP.add)
              for k, Wt in ((0, W_m1), (1, W_m2)):
                  for i in range(4):
                      t = p3.tile([128, NCH, NW], F32, tag="cmb",
                                  name=f"cmbm_{k}_{i}")
                      nc.vector.tensor_mul(t[:], Wt[:, :, 0:NW, 0],
                                           imb[:, i, None, :].broadcast_to(
                                               [128, NCH, NW]))
                      nc.vector.tensor_add(t[:], t[:],
                                           inb[:, i, None, :].broadcast_to(
                                               [128, NCH, NW]))
                      nc.vector.tensor_reduce(stat_m[:, :, k, i:i + 1], t[:],
                                              mybir.AxisListType.X, OP.max)
              nc.sync.dma_start(csum_in[:],
                                stat_s[:].rearrange("p a b c -> p (a b c)"))
              nc.sync.dma_start(cmax_in[:],
                                stat_m[:].rearrange("p a b c -> p (a b c)"))
              nc.gpsimd.collective_compute("AllReduce", OP.add,
                                           replica_groups=[core_ids],
                                           ins=[csum_in[:]], outs=[csum_out[:]])
              nc.gpsimd.collective_compute("AllReduce", OP.max,
                                           replica_groups=[core_ids],
                                           ins=[cmax_in[:]], outs=[cmax_out[:]])

              # ---- spatial sw path (PE work overlapping collectives) ----
              col0 = 0
              for ti, tl in enumerate(P34_TILES):
                  cs = slice(col0, col0 + tl)
                  zpsl = [zps.tile([128, tl], F32, tag="zp",
                                   name=f"zp_{col0}_{m}") for m in range(NCH)]
                  for mc in range(NCH):
                      for kc in range(2 * NCH):
                          rhs = (rgb_all[:, kc, cs] if kc < NCH
                                 else x_all[:, kc - NCH, cs])
                          nc.tensor.matmul(zpsl[mc][:],
                                           sw1wT[:, kc, mc * 128:(mc + 1) * 128],
                                           rhs, start=(kc == 0),
                                           stop=(kc == 2 * NCH - 1))
                  z_r = p3f.tile([128, NCH, tl], BF, tag="z_r", name=f"zr_{col0}")
                  for mc in range(NCH):
                      nc.scalar.activation(z_r[:, mc, :], zpsl[mc][:], AF.Relu,
                                           bias=sw1b_t[:, mc:mc + 1], scale=1.0)
                  sps_t = sps.tile([2, tl], F32, tag="sp", name=f"sp_{col0}")
                  for kc in range(NCH):
                      nc.tensor.matmul(sps_t[:], sw2wT[:, kc, :], z_r[:, kc, :],
                                       start=(kc == 0), stop=(kc == NCH - 1))
                  nc.vector.tensor_scalar_add(s01_all[:, cs], sps_t[:],
                                              sw2b_t[:])
                  col0 += tl
              # sigmoid(s)/2, split row 1 to partition 0
              nc.scalar.activation(s01_all[:], s01_all[:], AF.Sigmoid)
              nc.vector.tensor_scalar_mul(s01_all[:], s01_all[:], 0.5)
              nc.sync.dma_start(s1_row[:], s01_all[1:2, :])

              # channel MLP (sharded): ycat rhs [128, 24, 4]
              ycat_f = p3a.tile([128, 24, 4], F32)
              cso4 = csum_out[:].rearrange("p (c k i) -> p c k i", k=2, i=4)
              cmo4 = cmax_out[:].rearrange("p (c k i) -> p c k i", k=2, i=4)
              nc.sync.dma_start(ycat_f[:, 0:6, :], cso4[:, :, 0, :])
              nc.sync.dma_start(ycat_f[:, 6:12, :], cso4[:, :, 1, :])
              nc.sync.dma_start(ycat_f[:, 12:18, :], cmo4[:, :, 0, :])
              nc.sync.dma_start(ycat_f[:, 18:24, :], cmo4[:, :, 1, :])
              ycat_r = p3a.tile([128, 24, 4], BF)
              nc.vector.tensor_copy(ycat_r[:], ycat_f[:])
              z1sb = p3a.tile([128, 3, 4], BF)
              for mc in range(3):
                  ps = sps.tile([128, 4], F32, tag="sp", name=f"z1ps_{mc}")
                  for kc in range(24):
                      nc.tensor.matmul(ps[:], cw1wTs[:, kc, mc * 128:(mc + 1) * 128],
                                       ycat_r[:, kc, :], start=(kc == 0),
                                       stop=(kc == 23))
                  nc.scalar.activation(z1sb[:, mc, :], ps[:], AF.Relu,
                                       bias=cw1bs_t[:, mc:mc + 1], scale=1.0)
              z1f32 = p3a.tile([128, 3, 4], F32)
              nc.vector.tensor_copy(z1f32[:], z1sb[:])
              nc.sync.dma_start(z1_in[:].rearrange("(m p) f -> p m f", p=128),
                                z1f32[:])
              nc.gpsimd.collective_compute("AllGather", OP.bypass,
                                           replica_groups=[core_ids],
                                           ins=[z1_in[:]], outs=[z1_out[:]])
              z1f = p3a.tile([128, 24, 4], F32)
              nc.sync.dma_start(z1f[:],
                                z1_out[:].rearrange("(c p) f -> p c f", p=128))
              z1r = p3a.tile([128, 24, 4], BF)
              nc.vector.tensor_copy(z1r[:], z1f[:])
              z2sb = p3a.tile([128, 2, 4], F32)
              nc.any.memset(z2sb[:], 0.0)
              for mc, msz in ((0, 128), (1, 64)):
                  ps = sps.tile([128, 4], F32, tag="sp", name=f"z2ps_{mc}")
                  for kc in range(24):
                      nc.tensor.matmul(ps[0:msz, :],
                                       cw2wTs[:, kc, mc * 128:mc * 128 + msz],
                                       z1r[:, kc, :], start=(kc == 0),
                                       stop=(kc == 23))
                  nc.vector.tensor_scalar_add(z2sb[0:msz, mc, :], ps[0:msz, :],
                                              cw2bs_t[0:msz, mc:mc + 1])
              nc.sync.dma_start(z2_in[0:128, :], z2sb[:, 0, :])
              nc.sync.dma_start(z2_in[128:192, :], z2sb[0:64, 1, :])
              nc.gpsimd.collective_compute("AllGather", OP.bypass,
                                           replica_groups=[core_ids],
                                           ins=[z2_in[:]], outs=[z2_out[:]])
              y_f = p3a.tile([128, 12, 4], F32)
              nc.sync.dma_start(y_f[:],
                                z2_out[:].rearrange("(c p) f -> p c f", p=128))
              nc.scalar.activation(y_f[:], y_f[:], AF.Sigmoid)
              nc.vector.tensor_scalar_mul(y_f[:], y_f[:], 0.5)
              # transpose per chunk to [4, 128] bf16 for P4 matmuls
              cw0T = p3a.tile([4, NCH, 128], BF)   # y[:, :C]  (scales x1 -> out2)
              cw1T = p3a.tile([4, NCH, 128], BF)   # y[:, C:]  (scales x2 -> out1)
              for c in range(NCH):
                  for (dstt, src) in ((cw0T, y_f[:, c, :]),
                                      (cw1T, y_f[:, 6 + c, :])):
                      pstt = sps.tile([4, 128], F32, tag="sp",
                                      name=f"ct_{c}_{dstt.name}")
                      nc.tensor.transpose(pstt[:], src, ident128[:])
                      nc.scalar.copy(dstt[:, c, :], pstt[:])
              # rows of sw path as bf16 for rank-1 fold into P4 psums
              s0_bf = p3a.tile([1, NTOK], BF)
              nc.vector.tensor_copy(s0_bf[:], s01_all[0:1, :])
              s1_bf = p3a.tile([1, NTOK], BF)
              nc.vector.tensor_copy(s1_bf[:], s1_row[:])

              # ============ PHASE 4 (merged into phase-3 scope) ============
              p4 = p3
              cwp = zps
              col0 = 0
              for tl in P34_TILES:
                  cs = slice(col0, col0 + tl)
                  imsl = p4.tile([4, tl], BF, tag="imsr", name=f"ims_{col0}")
                  nc.sync.dma_start(imsl[:], imgsel_d[:, cs])
                  o1 = p4.tile([128, NCH, tl], BF, tag="o1", name=f"o1_{col0}")
                  o2 = p4.tile([128, NCH, tl], BF, tag="o2", name=f"o2_{col0}")
                  for c in range(NCH):
                      pc0 = cwp.tile([128, tl], F32, tag="zp",
                                     name=f"c0_{col0}_{c}")
                      nc.tensor.matmul(pc0[:], cw0T[:, c, :], imsl[:],
                                       start=True, stop=False)
                      nc.tensor.matmul(pc0[:], ones_row[:], s0_bf[:, cs],
                                       start=False, stop=True)
                      pc1 = cwp.tile([128, tl], F32, tag="zp",
                                     name=f"c1_{col0}_{c}")
                      nc.tensor.matmul(pc1[:], cw1T[:, c, :], imsl[:],
                                       start=True, stop=False)
                      nc.tensor.matmul(pc1[:], ones_row[:], s1_bf[:, cs],
                                       start=False, stop=True)
                      t0 = p4.tile([128, tl], BF, tag="t0", name=f"t0_{col0}_{c}")
                      nc.vector.tensor_mul(t0[:], pc1[:], x_all[:, c, cs])
                      nc.vector.tensor_add(o1[:, c, :], rgb_all[:, c, cs], t0[:])
                      t1 = p4.tile([128, tl], BF, tag="t1", name=f"t1_{col0}_{c}")
                      nc.vector.tensor_mul(t1[:], pc0[:], rgb_all[:, c, cs])
                      nc.vector.tensor_add(o2[:, c, :], x_all[:, c, cs], t1[:])
                  nc.sync.dma_start(r6(out1_d[:])[:, :, cs], o1[:])
                  nc.sync.dma_start(r6(out2_d[:])[:, :, cs], o2[:])
                  col0 += tl

        resp2_cm.__exit__(None, None, None)

    nc.compile()
    return nc


def _windowize(x):
    # [B, 64, 64, C] -> [104, 196, C] padded windows
    Bp = np.zeros((B, 70, 70, C), x.dtype)
    Bp[:, :64, :64, :] = x
    w = Bp.reshape(B, GRID, WIN, GRID, WIN, C).transpose(0, 1, 3, 2, 4, 5)
    w = w.reshape(NWIN_TOT, N, C)
    out = np.zeros((NCORE * NW, N, C), x.dtype)
    out[:NWIN_TOT] = w
    return out


def _unwindowize(perwin):
    # [104, 196, C] -> [B, 64, 64, C]
    w = perwin[:NWIN_TOT].reshape(B, GRID, GRID, WIN, WIN, C)
    w = w.transpose(0, 1, 3, 2, 4, 5).reshape(B, 70, 70, C)
    return np.ascontiguousarray(w[:, :64, :64, :])


def kernel(rgb_embedding, x_embedding, norm1_w, norm1_b, qkv_w, qkv_b,
           rel_pos_h, rel_pos_w, proj_w, proj_b, norm2_w, norm2_b,
           fc1_w, fc1_b, fc2_w, fc2_b, cw1_w, cw1_b, cw2_w, cw2_b,
           sw1_w, sw1_b, sw2_w, sw2_b):
    if "nc" not in _CACHE:
        _CACHE["nc"] = _build()
    nc = _CACHE["nc"]

    f32 = lambda a: np.ascontiguousarray(a, dtype=np.float32)
    bf = lambda a: np.ascontiguousarray(np.asarray(a, dtype=np.float32)
                                        .astype(NPBF))
    f8w = lambda a: np.ascontiguousarray(
        np.clip(np.asarray(a, np.float32), -240, 240)
        .astype(ml_dtypes.float8_e4m3))
    xw = _windowize(f32(x_embedding))        # [104, 196, C]
    rw = _windowize(f32(rgb_embedding))
    vm = np.zeros((NCORE * NW, N), np.float32)
    vh = np.minimum(np.maximum(64 - np.arange(GRID) * WIN, 0), WIN)
    wm = np.zeros((GRID, GRID, WIN, WIN), np.float32)
    for a in range(GRID):
        for b in range(GRID):
            wm[a, b, :vh[a], :vh[b]] = 1.0
    vm[:NWIN_TOT] = np.tile(wm.reshape(GRID * GRID, N), (B, 1))
    win_img = np.full(NCORE * NW, -1, np.int64)
    win_img[:NWIN_TOT] = np.arange(NWIN_TOT) // (GRID * GRID)

    qkb = f32(qkv_b[:2 * C]).copy()
    qkb[:C] *= SCALE
    tab = np.concatenate([f32(rel_pos_h)[::-1], f32(rel_pos_w)[::-1]], axis=0)
    tab = np.ascontiguousarray(tab.T) * (1.0 / SCALE)
    cw1s = f32(cw1_w).T.copy()              # [4C(k), 4C(m)]
    cw1s[:2 * C, :] *= 1.0 / (HH * WW)      # fold avg divisor
    shared = dict(
        qkwT=bf(f32(qkv_w)[:2 * C].T), qkb=qkb,
        vwT=bf(f32(qkv_w)[2 * C:].T), vb=f32(qkv_b[2 * C:]).copy(),
        projwT=bf(f32(proj_w).T), projb=f32(proj_b),
        n1w=f32(norm1_w), n1b=f32(norm1_b), n2w=f32(norm2_w), n2b=f32(norm2_b),
        tab=bf(tab),
        fc1w8=f8w(f32(fc1_w).T.reshape(3, 2, 128, DFF)
                  .transpose(2, 0, 1, 3)), fc1b=f32(fc1_b),
        fc2wT=bf(f32(fc2_w).T), fc2b=f32(fc2_b),
        sw1wT=bf(f32(sw1_w).T), sw1b=f32(sw1_b),
        sw2wT=bf(f32(sw2_w).T), sw2b=f32(sw2_b),
    )
    cw2s = np.ascontiguousarray(f32(cw2_w).T)  # [4C, 2C]
    in_maps = []
    for c in range(NCORE):
        sl = slice(c * NW, (c + 1) * NW)
        xT = bf(xw[sl].reshape(NTOK, C).T)
        rT = bf(rw[sl].reshape(NTOK, C).T)
        mrow = vm[sl].reshape(1, NTOK).copy()
        imgm = np.zeros((4, NW), np.float32)
        imsel = np.zeros((4, NTOK), np.float32)
        for wloc in range(NW):
            im = win_img[c * NW + wloc]
            if im >= 0:
                imgm[im, wloc] = 1.0
                imsel[im, wloc * N:(wloc + 1) * N] = 1.0
        m = dict(shared)
        m.update(
            xT=xT, rgbT=rT, mask=bf(mrow),
            imgmask=imgm, imgneg=(imgm - 1.0) * 1e30, imgsel=bf(imsel),
            cw1wTs=bf(cw1s[:, c * 384:(c + 1) * 384]),
            cw1bs=f32(cw1_b[c * 384:(c + 1) * 384]).copy(),
            cw2wTs=bf(cw2s[:, c * 192:(c + 1) * 192]),
            cw2bs=f32(cw2_b[c * 192:(c + 1) * 192]).copy(),
        )
        in_maps.append(m)

    trace = bool(os.environ.get("KERNEL_TRACE"))
    res = run_bass_kernel_spmd(nc, in_maps, list(range(NCORE)), trace=trace)
    if trace:
        _CACHE["exec_time_ns"] = res.exec_time_ns
    o1 = np.zeros((NCORE * NW, N, C), np.float32)
    o2 = np.zeros((NCORE * NW, N, C), np.float32)
    for c in range(NCORE):
        sl = slice(c * NW, (c + 1) * NW)
        o1[sl] = res.results[c]["out1T"].astype(np.float32).T.reshape(NW, N, C)
        o2[sl] = res.results[c]["out2T"].astype(np.float32).T.reshape(NW, N, C)
    rgb_out = _unwindowize(o1)
    x_out = _unwindowize(o2)
    return rgb_out, x_out
